# revision 13
# baseline (speedup 1.0000x reference)
"""Trainium2 Bass kernel for nn_ASTGATClassifier (3-layer GAT + BN + ELU + pool + MLP).

v3 strategy (8 NeuronCores, SPMD single program), built around the TimelineSim
cost model's pricing (DMA per-descriptor, collectives 15us const + out bytes):

  - Edges SRC-partitioned; GAT softmax division deferred past a bf16
    ReduceScatter of a [NTOT, OC+H] (numerator ++ denominator) accumulator.
  - Self-loops are REMOVED from the edge stream (they concentrated 128
    slots/window on the dst-owner core, inflating the uniform per-window
    slot budget by ~55%) and folded in algebraically after the RS:
    out = (num_rs + exp_self*x_i) / (den_rs + exp_self), with
    exp_self = exp(leaky(als_i + ald_i)) computed core-locally.
  - K_CH=2 RS chunks (15us constant per collective), NBLK=50 (N2=6400).
  - Per-window slot counts are the exact max over cores (no ceil16);
    (chunk, ald-half) runs still pad to 128.
  - Scatter-add via one-hot matmuls into per-128-dst-window PSUM; windows
    flush through a 5-window bf16 stage; region DMAs feed the chunk RS.
  - al_d travels via a tiny AllGather of [N2,4] + DRAM expansion into 256B
    rows for the per-edge gather.  BN stats via ones-matmuls + AllReduce,
    with pad rows masked.  Pooling + classifier as before.
"""

import sys

sys.path.insert(0, "/opt/trn_rl_repo")

import numpy as np
import ml_dtypes

N_NODES = 50000
N_EDGES = 400000
N_GRAPHS = 256
NUM_TYPES = 200
EMB = 64
HID = 128
HEADS = 4
GDIM = 256
NUM_CLASSES = 20
EPS = 1e-5
NEG = 0.2
EPS_DEN = 1e-20

NC = 8
NSH = N_NODES // NC          # 6250 nodes per core
NBLK = 50                    # node blocks per core (50*128 = 6400)
N2 = NBLK * 128              # padded shard
NTOT = NC * N2               # 51200
K_CH = 2                     # RS chunks
CH = N2 // K_CH              # 3200 rows per (chunk, core) region
RB = CH // 128               # 25 windows per region
NW = K_CH * NC * RB          # 400 windows
ALD_HALF = (NC // 2) * N2    # 25600
GRP = 5                      # windows per stage/flush group

# per-layer config: (IN, OC, H, EW(gather row cols), AC(accum cols))
LCFG = [
    (EMB, 512, 4, 640, 516),
    (512, 512, 4, 640, 516),
    (512, 128, 1, 256, 129),
]
SPAN_SLOTS = 1792            # max slots per gather call (14 tiles)

BF16 = ml_dtypes.bfloat16

_CACHE = {}


def _wrap_idx(idx):
    """int16 gather index layout: [128, n/16]; idx j at [j%16, j//16], tiled x8."""
    n = len(idx)
    assert n % 16 == 0
    a = np.asarray(idx, dtype=np.int16).reshape(n // 16, 16).T
    return np.tile(a, (8, 1))


def preprocess(x, edge_index, depth, batch):
    """Host-side index preprocessing -> per-core blobs + uniform schedule."""
    x = np.asarray(x).astype(np.int64)
    ei = np.asarray(edge_index).astype(np.int64)
    batch = np.asarray(batch).astype(np.int64)
    src = ei[0]
    dst = ei[1]

    # destination-side row mappings (global)
    oc = dst // NSH
    locd = dst - oc * NSH
    kch = locd // CH
    arow = kch * (NC * CH) + oc * CH + (locd - kch * CH)   # accum row (chunk-major)
    wind = arow // 128
    d128 = arow % 128
    aldrow = oc * N2 + locd                                 # ald table row
    half = (oc >= NC // 2).astype(np.int64)
    aldidx = aldrow - half * ALD_HALF

    core_of_src = src // NSH
    percore = []
    cnts = np.zeros((NC, NW), dtype=np.int64)
    for c in range(NC):
        m = core_of_src == c
        sl = (src[m] - c * NSH).astype(np.int64)
        wc, ac, dc, aic = wind[m], arow[m], d128[m], aldidx[m]
        order = np.argsort(ac, kind="stable")
        sl, wc, dc, aic = sl[order], wc[order], dc[order], aic[order]
        percore.append((sl, wc, dc, aic))
        cnts[c] = np.bincount(wc, minlength=NW)

    slots_w = cnts.max(axis=0).astype(np.int64)   # exact max, no ceil

    # build slot stream: windows in order; pad each (k, half) run to %128
    slot_w = []          # per-slot window id (-1 = run pad)
    ws_start = np.zeros(NW, dtype=np.int64)
    run_bounds = []      # (slot_lo, slot_hi, half) per run
    pos = 0
    for k in range(K_CH):
        for hf in range(2):
            run_lo = pos
            w0 = k * NC * RB + hf * (NC // 2) * RB
            for w in range(w0, w0 + (NC // 2) * RB):
                ws_start[w] = pos
                slot_w.extend([w] * int(slots_w[w]))
                pos += int(slots_w[w])
            r = (-pos) % 128
            slot_w.extend([-1] * r)
            pos += r
            run_bounds.append((run_lo, pos, hf))
    S = pos
    assert S % 128 == 0
    slot_w = np.asarray(slot_w, dtype=np.int64)
    n_tiles = S // 128

    # spans: cut runs into <= SPAN_SLOTS pieces (128-aligned)
    spans = []
    for (lo, hi, hf) in run_bounds:
        p = lo
        while p < hi:
            n = min(SPAN_SLOTS, hi - p)
            spans.append((p, n, hf))
            p += n

    # pairs: per tile, windows overlapping its slot range
    ws_end = ws_start + slots_w
    pairs = []            # (tile, w, first, last)
    pairs_of_tile = [[] for _ in range(n_tiles)]
    for w in range(NW):
        if slots_w[w] == 0:
            continue
        t0 = int(ws_start[w] // 128)
        t1 = int((ws_end[w] - 1) // 128)
        for t in range(t0, t1 + 1):
            pc = len(pairs)
            pairs.append([t, w, t == t0, t == t1])
            pairs_of_tile[t].append(pc)
    NPAIR = len(pairs)

    # per-span contiguous pair-column range (pairs are (w, t)-lex ordered,
    # which matches slot order, so each span's pairs are contiguous)
    span_pc = []
    for (s0, ns, hf) in spans:
        pcs = []
        for t in range(s0 // 128, (s0 + ns) // 128):
            pcs.extend(pairs_of_tile[t])
        assert pcs == list(range(pcs[0], pcs[-1] + 1))
        assert len(pcs) <= 4 * (SPAN_SLOTS // 128)
        span_pc.append((pcs[0], pcs[-1] + 1))

    # per (region, group): the last non-empty window triggers the group flush
    # group id = (k, oc, g);  windows w0+g*GRP .. w0+(g+1)*GRP-1
    flush_trigger = {}    # w -> list of (k, oc, g) groups it closes
    for k in range(K_CH):
        for occ in range(NC):
            r0 = (k * NC + occ) * RB
            for g in range(RB // GRP):
                ws = [r0 + g * GRP + j for j in range(GRP)]
                nonempty = [w for w in ws if slots_w[w] > 0]
                assert nonempty, "fully-empty flush group"
                flush_trigger.setdefault(nonempty[-1], []).append((k, occ, g))

    # per-core blobs
    blobs = []
    for c in range(NC):
        sl, wc, dc, aic = percore[c]
        xidx = np.zeros(S, dtype=np.int64)
        aidx = np.zeros(S, dtype=np.int64)
        dloc = np.full(S, -1.0, dtype=np.float32)
        offs = ws_start[wc]
        within = np.zeros(len(wc), dtype=np.int64)
        uw, uidx, ucnt = np.unique(wc, return_index=True, return_counts=True)
        for u, i0, cc in zip(uw, uidx, ucnt):
            within[i0 : i0 + cc] = np.arange(cc)
        slot = offs + within
        xidx[slot] = sl
        aidx[slot] = aic
        dloc[slot] = dc.astype(np.float32)
        # dstloc blob per pair
        dl = np.full((128, NPAIR), -1.0, dtype=np.float32)
        for pc, (t, w, fi, la) in enumerate(pairs):
            seg = dloc[t * 128 : (t + 1) * 128].copy()
            m = slot_w[t * 128 : (t + 1) * 128] != w
            seg[m] = -1.0
            dl[:, pc] = seg
        blobs.append(
            dict(
                xidx=_wrap_idx(xidx),
                aldidx=_wrap_idx(aidx),
                dstloc=dl,
            )
        )

    # emb gather idx + depth rows per core (layer-0 prolog) + pad mask
    mask = np.zeros((128, NBLK), dtype=np.float32)
    for nt in range(NBLK):
        mask[:, nt] = (nt * 128 + np.arange(128)) < NSH
    for c in range(NC):
        ids = np.zeros(N2, dtype=np.int64)
        ids[:NSH] = x[c * NSH : (c + 1) * NSH]
        blobs[c]["emb_idx"] = _wrap_idx(ids)
        dr = np.zeros((1, N2), dtype=np.float32)
        dr[0, :NSH] = np.asarray(depth, dtype=np.float32)[c * NSH : (c + 1) * NSH]
        blobs[c]["depth_row"] = dr
        blobs[c]["maskcol"] = mask

    # pooling segments
    counts = np.bincount(batch, minlength=N_GRAPHS)
    starts = np.concatenate([[0], np.cumsum(counts)])
    segs = []
    for cc in range(NC):
        lo_n, hi_n = cc * NSH, (cc + 1) * NSH
        lst = []
        for g in range(N_GRAPHS):
            a, bnd = starts[g], starts[g + 1]
            aa, bb = max(a, lo_n), min(bnd, hi_n)
            if aa < bb:
                lst.append((int(aa - lo_n), int(bb - lo_n), int(g), float(1.0 / max(counts[g], 1))))
        segs.append(lst)
    for c in range(NC):
        m8 = np.zeros((128, NC), dtype=np.float32)
        m8[:, c] = 1.0
        m8n = np.where(m8 > 0, 0.0, -1e30).astype(np.float32)
        blobs[c]["mask8"] = m8
        blobs[c]["mask8n"] = m8n

    sched = dict(
        slots_w=slots_w, ws_start=ws_start, ws_end=ws_end, spans=spans,
        pairs=pairs, pairs_of_tile=pairs_of_tile, n_tiles=n_tiles, S=S,
        NPAIR=NPAIR, segs=segs, flush_trigger=flush_trigger, span_pc=span_pc,
    )
    return dict(sched=sched, blobs=blobs)


def build_param_blobs(p):
    """Host-side parameter layout transforms (bf16 casts, folds, transposes)."""
    f32 = np.float32
    out = {}

    def fold_a(W, a_s, a_d, heads, c):
        W3 = W.reshape(heads, c, -1)
        As = np.einsum("hck,hc->kh", W3, a_s).astype(f32)
        Ad = np.einsum("hck,hc->kh", W3, a_d).astype(f32)
        return np.concatenate([As, Ad], axis=1)  # [IN, 2H]

    out["w0x"] = np.ascontiguousarray(p["W0"].T).astype(BF16)
    out["w0al"] = fold_a(p["W0"], p["as0"], p["ad0"], HEADS, HID).astype(BF16)
    out["w1x"] = np.ascontiguousarray(p["W1"].T).astype(BF16)
    out["w1al"] = fold_a(p["W1"], p["as1"], p["ad1"], HEADS, HID).astype(BF16)
    out["w2x"] = np.ascontiguousarray(p["W2"].T).astype(BF16)
    out["w2al"] = fold_a(p["W2"], p["as2"], p["ad2"], 1, GDIM // 2).astype(BF16)
    out["emb_t"] = np.asarray(p["emb_table"], dtype=f32)
    out["dw_row"] = np.asarray(p["depth_w"], dtype=f32).reshape(1, EMB)
    out["db_row"] = np.asarray(p["depth_b"], dtype=f32).reshape(1, EMB)
    for l, (g, be) in enumerate([(p["g0"], p["be0"]), (p["g1"], p["be1"]), (p["g2"], p["be2"])]):
        out[f"gam{l}"] = np.asarray(g, dtype=f32).reshape(1, -1)
        out[f"bet{l}"] = np.asarray(be, dtype=f32).reshape(1, -1)
    out["cw1t"] = np.ascontiguousarray(p["cw1"].T).astype(f32)
    out["cb1c"] = np.asarray(p["cb1"], dtype=f32).reshape(2, 128).T.copy()
    out["cw2t"] = np.ascontiguousarray(p["cw2"].T).astype(f32)
    out["cb2c"] = np.asarray(p["cb2"], dtype=f32).reshape(NUM_CLASSES, 1)
    out["iotab"] = np.tile(np.arange(128, dtype=f32)[None, :], (128, 1)).astype(BF16)
    out["iden_f"] = np.eye(128, dtype=f32)
    out["iden_b"] = np.eye(128).astype(BF16)
    out["ones_b"] = np.ones((128, 1), dtype=BF16)
    out["ones_r"] = np.ones((1, 128), dtype=f32)
    return out


def build_nc(pre):
    """Trace the full SPMD bass program (structure from `pre['sched']`)."""
    import concourse.bacc as bacc
    import concourse.bass as bass
    import concourse.mybir as mybir
    import concourse.tile as tile
    from concourse.library_config import mlp
    from contextlib import ExitStack

    dt = mybir.dt
    ALU = mybir.AluOpType
    ACTF = mybir.ActivationFunctionType
    AXX = mybir.AxisListType.X

    sch = pre["sched"]
    spans = sch["spans"]
    pairs = sch["pairs"]
    pairs_of_tile = sch["pairs_of_tile"]
    n_tiles = sch["n_tiles"]
    S = sch["S"]
    NPAIR = sch["NPAIR"]
    slots_w = sch["slots_w"]
    segs = sch["segs"]
    flush_trigger = sch["flush_trigger"]
    span_pc = sch["span_pc"]

    nc = bacc.Bacc("TRN2", target_bir_lowering=False, debug=False, num_devices=NC)

    b0 = pre["blobs"][0]
    EIN = {}

    def ein(name, arr_like, dtyp):
        EIN[name] = nc.dram_tensor(name, list(arr_like.shape), dtyp, kind="ExternalInput").ap()
        return EIN[name]

    i_xidx = ein("xidx", b0["xidx"], dt.int16)
    i_aldidx = ein("aldidx", b0["aldidx"], dt.int16)
    i_dstloc = ein("dstloc", b0["dstloc"], dt.float32)
    i_embidx = ein("emb_idx", b0["emb_idx"], dt.int16)
    i_depth = ein("depth_row", b0["depth_row"], dt.float32)
    i_mask8 = ein("mask8", b0["mask8"], dt.float32)
    i_mask8n = ein("mask8n", b0["mask8n"], dt.float32)
    i_maskcol = ein("maskcol", b0["maskcol"], dt.float32)
    P = {}
    P["w0x"] = ein("w0x", np.zeros((EMB, 512)), dt.bfloat16)
    P["w0al"] = ein("w0al", np.zeros((EMB, 8)), dt.bfloat16)
    P["w1x"] = ein("w1x", np.zeros((512, 512)), dt.bfloat16)
    P["w1al"] = ein("w1al", np.zeros((512, 8)), dt.bfloat16)
    P["w2x"] = ein("w2x", np.zeros((512, 128)), dt.bfloat16)
    P["w2al"] = ein("w2al", np.zeros((512, 2)), dt.bfloat16)
    P["emb_t"] = ein("emb_t", np.zeros((NUM_TYPES, EMB)), dt.float32)
    P["dw_row"] = ein("dw_row", np.zeros((1, EMB)), dt.float32)
    P["db_row"] = ein("db_row", np.zeros((1, EMB)), dt.float32)
    for l, ocl in [(0, 512), (1, 512), (2, 128)]:
        P[f"gam{l}"] = ein(f"gam{l}", np.zeros((1, ocl)), dt.float32)
        P[f"bet{l}"] = ein(f"bet{l}", np.zeros((1, ocl)), dt.float32)
    P["cw1t"] = ein("cw1t", np.zeros((GDIM, GDIM)), dt.float32)
    P["cb1c"] = ein("cb1c", np.zeros((128, 2)), dt.float32)
    P["cw2t"] = ein("cw2t", np.zeros((GDIM, NUM_CLASSES)), dt.float32)
    P["cb2c"] = ein("cb2c", np.zeros((NUM_CLASSES, 1)), dt.float32)
    P["iotab"] = ein("iotab", np.zeros((128, 128)), dt.bfloat16)
    P["iden_f"] = ein("iden_f", np.zeros((128, 128)), dt.float32)
    P["iden_b"] = ein("iden_b", np.zeros((128, 128)), dt.bfloat16)
    P["ones_b"] = ein("ones_b", np.zeros((128, 1)), dt.bfloat16)
    P["ones_r"] = ein("ones_r", np.zeros((1, 128)), dt.float32)

    out_dram = nc.dram_tensor("out", [N_GRAPHS, NUM_CLASSES], dt.float32, kind="ExternalOutput").ap()

    with tile.TileContext(nc) as tc, ExitStack() as stk:
        nc.gpsimd.load_library(mlp)
        sb = stk.enter_context(tc.tile_pool(name="sb", bufs=2))
        sb1 = stk.enter_context(tc.tile_pool(name="sb1", bufs=1))
        ps = stk.enter_context(tc.tile_pool(name="ps", bufs=2, space="PSUM"))
        dram = stk.enter_context(tc.tile_pool(name="dram", bufs=1, space="DRAM"))

        def load_sb(ap, shape, dtyp, tag, pool=sb1):
            t = pool.tile(shape, dtyp, tag=tag)
            nc.sync.dma_start(t[:], ap[:, :])
            return t

        iota_sb = load_sb(P["iotab"], [128, 128], dt.bfloat16, "iota")
        idenb_sb = load_sb(P["iden_b"], [128, 128], dt.bfloat16, "idenb")
        idenf_sb = load_sb(P["iden_f"], [128, 128], dt.float32, "idenf")
        onesb_sb = load_sb(P["ones_b"], [128, 1], dt.bfloat16, "onesb")
        onesr_sb = load_sb(P["ones_r"], [1, 128], dt.float32, "onesr")
        dw_sb = load_sb(P["dw_row"], [1, EMB], dt.float32, "dwrow")
        db_sb = load_sb(P["db_row"], [1, EMB], dt.float32, "dbrow")
        mask_sb = load_sb(i_maskcol, [128, NBLK], dt.float32, "maskcol")

        wx_sb = {}
        wal_sb = {}
        for l, (IN, OC, H, EW, AC) in enumerate(LCFG):
            nch_in = (IN + 127) // 128
            wx_sb[l] = []
            wal_sb[l] = []
            wxn, waln = f"w{l}x", f"w{l}al"
            for k in range(nch_in):
                kp = min(IN - k * 128, 128)
                tx = sb1.tile([kp, OC], dt.bfloat16, tag=f"wx{l}_{k}")
                nc.sync.dma_start(tx[:], P[wxn][k * 128 : k * 128 + kp, :])
                wx_sb[l].append(tx)
                ta = sb1.tile([kp, 2 * H], dt.bfloat16, tag=f"wal{l}_{k}")
                nc.sync.dma_start(ta[:], P[waln][k * 128 : k * 128 + kp, :])
                wal_sb[l].append(ta)

        # persistent transposed features (lhsT for x_phase; final = pooling input)
        hTe = sb1.tile([128, 4 * N2], dt.bfloat16, tag="hTe")

        # DRAM tensors
        x_tbl = dram.tile([N2, 640], dt.bfloat16, tag="x_tbl")
        ald_in = dram.tile([N2, 4], dt.bfloat16, tag="ald_in")
        ald_fulls = [
            dram.tile([NTOT, 4], dt.bfloat16, tag=f"ald_full{l}", addr_space="Shared", name=f"ald_full{l}")
            for l in range(3)
        ]
        ald_tbl = dram.tile([NTOT, 128], dt.bfloat16, tag="ald_tbl")
        accA = dram.tile([NTOT, 516], dt.bfloat16, tag="accA")
        accC = dram.tile([NTOT, 129], dt.bfloat16, tag="accC")
        rsA = dram.tile([N2, 516], dt.bfloat16, tag="rsA")
        rsC = dram.tile([N2, 129], dt.bfloat16, tag="rsC")

        # =========================================================
        # Layer-0 prolog: hTe[0:64] = (emb + depth-proj)^T
        # =========================================================
        embidx_sb = load_sb(i_embidx, [128, N2 // 16], dt.int16, "embidx")
        emb_g = sb.tile([128, NBLK * EMB], dt.float32, tag="xg", name="emb_g")
        nc.gpsimd.dma_gather(
            emb_g[:].rearrange("p (t w) -> p t w", w=EMB),
            P["emb_t"][:, :],
            embidx_sb[:],
            N2, N2, EMB, single_packet=False,
        )
        for j5 in range(NBLK // 5):
            dr5 = sb.tile([1, 5 * 128], dt.float32, tag="dr5", bufs=1)
            nc.sync.dma_start(dr5[:], i_depth[0:1, j5 * 640 : (j5 + 1) * 640])
            for j in range(5):
                nt = j5 * 5 + j
                ps_t = ps.tile([EMB, 128], dt.float32, tag="med")
                nc.tensor.matmul(out=ps_t[:], lhsT=dw_sb[:], rhs=dr5[0:1, j * 128 : (j + 1) * 128], start=True, stop=False)
                nc.tensor.matmul(out=ps_t[:], lhsT=db_sb[:], rhs=onesr_sb[:], start=False, stop=False)
                nc.tensor.matmul(
                    out=ps_t[:],
                    lhsT=emb_g[:, nt * EMB : (nt + 1) * EMB],
                    rhs=idenf_sb[:],
                    is_transpose=True,
                    start=False,
                    stop=True,
                )
                nc.vector.tensor_copy(hTe[0:EMB, nt * 128 : (nt + 1) * 128], ps_t[:])

        # =========================================================
        # per-layer phases
        # =========================================================
        def x_phase(l):
            IN, OC, H, EW, AC = LCFG[l]
            nch = (IN + 127) // 128

            def lhs(k, nt):
                kp = min(IN - k * 128, 128)
                return hTe[0:kp, k * N2 + nt * 128 : k * N2 + (nt + 1) * 128]

            # pass A: attention rows (al_s, al_d kept in SBUF; al_d staged + AllGather)
            als_sb = sb.tile([128, NBLK * 4], dt.float32, tag="als", bufs=1)
            alds = sb.tile([128, NBLK * 4], dt.bfloat16, tag="alds", bufs=1)
            nc.vector.memset(als_sb[:], 0)
            nc.vector.memset(alds[:], 0)
            for nt in range(NBLK):
                ps_al = ps.tile([128, 8], dt.float32, tag="small")
                for k in range(nch):
                    nc.tensor.matmul(out=ps_al[:, 0 : 2 * H], lhsT=lhs(k, nt), rhs=wal_sb[l][k][:],
                                     start=(k == 0), stop=(k == nch - 1))
                nc.vector.tensor_copy(als_sb[:, nt * 4 : nt * 4 + H], ps_al[:, 0:H])
                nc.vector.tensor_copy(alds[:, nt * 4 : nt * 4 + H], ps_al[:, H : 2 * H])
            nc.sync.dma_start(
                ald_in[:, :].rearrange("(t p) w -> p t w", p=128), alds[:].rearrange("p (t w) -> p t w", w=4)
            )
            nc.gpsimd.collective_compute(
                "AllGather", ALU.bypass, ins=[ald_in[:, :]], outs=[ald_fulls[l][:, :]],
                replica_groups=[list(range(NC))],
            )
            nc.sync.dma_start(ald_tbl[:, 0:4], ald_fulls[l][:, :])

            # self-loop exp: exps = exp(leaky(als_i + ald_i))  [128, NBLK*4] f32
            exps = sb.tile([128, NBLK * 4], dt.float32, tag="exps", bufs=1)
            nc.vector.tensor_tensor(out=exps[:], in0=als_sb[:], in1=alds[:], op=ALU.add)
            zneg = sb.tile([128, NBLK * 4], dt.float32, tag="zneg", bufs=1)
            nc.vector.tensor_scalar(out=zneg[:], in0=exps[:], scalar1=NEG, scalar2=None, op0=ALU.mult)
            nc.vector.tensor_tensor(out=exps[:], in0=exps[:], in1=zneg[:], op=ALU.max)
            nc.scalar.activation(exps[:], exps[:], ACTF.Exp)

            # pass B: x rows -> x_tbl (overlaps AllGather)
            for nt in range(NBLK):
                ps_x = ps.tile([128, 512], dt.float32, tag="win", bufs=2)
                for k in range(nch):
                    nc.tensor.matmul(out=ps_x[:, 0:OC], lhsT=lhs(k, nt), rhs=wx_sb[l][k][:],
                                     start=(k == 0), stop=(k == nch - 1))
                xa_t = sb.tile([128, EW], dt.bfloat16, tag="xa_t", bufs=2)
                nc.scalar.activation(xa_t[:, 0:OC], ps_x[:, 0:OC], ACTF.Copy)
                nc.vector.tensor_copy(xa_t[:, OC : OC + H], als_sb[:, nt * 4 : nt * 4 + H])
                nc.sync.dma_start(x_tbl[nt * 128 : (nt + 1) * 128, 0:EW], xa_t[:])
            return exps

        def edge_phase(l):
            IN, OC, H, EW, AC = LCFG[l]
            acc = accA if l < 2 else accC
            rs_out = rsA if l < 2 else rsC
            ST = SPAN_SLOTS // 128
            open_ps = {}
            stage = {"t": None}

            def flush(w, psw):
                ps_o, ps_d = psw
                r, b = w // RB, w % RB
                g, bj = b // GRP, b % GRP
                if stage["t"] is None:
                    stage["t"] = sb.tile([128, GRP * AC], dt.bfloat16, tag="stage", bufs=2, name="stage_t")
                    # zero slots of empty windows in this group (they never flush)
                    r0 = r * RB + g * GRP
                    for j in range(GRP):
                        if slots_w[r0 + j] == 0:
                            nc.vector.memset(stage["t"][:, j * AC : (j + 1) * AC], 0)
                dstc = stage["t"][:, bj * AC : bj * AC + OC]
                if (w % 2) == 0:
                    nc.scalar.activation(dstc, ps_o[:, 0:OC], ACTF.Copy)
                else:
                    nc.vector.tensor_copy(dstc, ps_o[:, 0:OC])
                nc.vector.tensor_copy(stage["t"][:, bj * AC + OC : bj * AC + OC + H], ps_d[:, 0:H])
                for (k, occ, gg) in flush_trigger.get(w, ()):
                    rows0 = k * (NC * CH) + occ * CH + gg * GRP * 128
                    nc.sync.dma_start(
                        acc[rows0 : rows0 + GRP * 128, :].rearrange("(t p) w -> p t w", p=128),
                        stage["t"][:].rearrange("p (t w) -> p t w", w=AC),
                    )
                    stage["t"] = None
                    if occ == NC - 1 and gg == RB // GRP - 1:
                        nc.gpsimd.collective_compute(
                            "ReduceScatter", ALU.add,
                            ins=[acc[k * NC * CH : (k + 1) * NC * CH, :]],
                            outs=[rs_out[k * CH : (k + 1) * CH, :]],
                            replica_groups=[list(range(NC))],
                        )

            for si, (s0, ns, hf) in enumerate(spans):
                t0 = s0 // 128
                nt = ns // 128
                pc_lo, pc_hi = span_pc[si]
                xi_t = sb.tile([128, SPAN_SLOTS // 16], dt.int16, tag="xi", bufs=2)
                nc.sync.dma_start(xi_t[:, 0 : ns // 16], i_xidx[:, s0 // 16 : (s0 + ns) // 16])
                ai_t = sb.tile([128, SPAN_SLOTS // 16], dt.int16, tag="ai", bufs=2)
                nc.sync.dma_start(ai_t[:, 0 : ns // 16], i_aldidx[:, s0 // 16 : (s0 + ns) // 16])
                dl_t = sb.tile([128, 4 * ST], dt.float32, tag="dl", bufs=2)
                nc.sync.dma_start(dl_t[:, 0 : pc_hi - pc_lo], i_dstloc[:, pc_lo:pc_hi])
                xg = sb.tile([128, ST * EW], dt.bfloat16, tag="xg", bufs=2)
                xg3 = xg[:, 0 : nt * EW].rearrange("p (t w) -> p t w", w=EW)
                nc.gpsimd.dma_gather(
                    xg3, x_tbl[:, 0:EW], xi_t[:, 0 : ns // 16],
                    ns, ns, EW, elem_step=640, single_packet=False,
                )
                aldg = sb.tile([128, ST * 128], dt.bfloat16, tag="aldg", bufs=2)
                aldg3 = aldg[:, 0 : nt * 128].rearrange("p (t w) -> p t w", w=128)
                nc.gpsimd.dma_gather(
                    aldg3, ald_tbl[hf * ALD_HALF : hf * ALD_HALF + ALD_HALF, :],
                    ai_t[:, 0 : ns // 16],
                    ns, ns, 128, single_packet=False,
                )
                # logits -> exp over the span
                zt = sb.tile([128, ST * 4], dt.float32, tag="zt", bufs=3)
                nc.vector.tensor_tensor(
                    out=zt[:, 0 : nt * H].rearrange("p (t h) -> p t h", h=H),
                    in0=xg3[:, :, OC : OC + H], in1=aldg3[:, :, 0:H], op=ALU.add,
                )
                z2 = sb.tile([128, ST * 4], dt.float32, tag="z2", bufs=3)
                nc.vector.tensor_scalar(out=z2[:, 0 : nt * H], in0=zt[:, 0 : nt * H],
                                        scalar1=NEG, scalar2=None, op0=ALU.mult)
                nc.vector.tensor_tensor(out=z2[:, 0 : nt * H], in0=zt[:, 0 : nt * H],
                                        in1=z2[:, 0 : nt * H], op=ALU.max)
                ex = sb.tile([128, ST * 4], dt.float32, tag="ex", bufs=3)
                nc.scalar.activation(ex[:, 0 : nt * H], z2[:, 0 : nt * H], ACTF.Exp)
                exb = sb.tile([128, ST * 4], dt.bfloat16, tag="exb", bufs=3)
                nc.vector.tensor_copy(exb[:, 0 : nt * H], ex[:, 0 : nt * H])
                for tl in range(nt):
                    t = t0 + tl
                    plist = pairs_of_tile[t]
                    if not plist:
                        continue
                    xgs = sb.tile([128, 512], dt.bfloat16, tag="xgs", bufs=6)
                    for h in range(H):
                        nc.vector.tensor_scalar(
                            out=xgs[:, h * 128 : (h + 1) * 128],
                            in0=xg3[:, tl, h * 128 : (h + 1) * 128],
                            scalar1=ex[:, tl * H + h : tl * H + h + 1],
                            scalar2=None, op0=ALU.mult,
                        )
                    for pc in plist:
                        _, w, first, last = pairs[pc]
                        oh = sb.tile([128, 128], dt.bfloat16, tag="oh", bufs=8)
                        nc.vector.tensor_scalar(
                            out=oh[:], in0=iota_sb[:], scalar1=dl_t[:, pc - pc_lo : pc - pc_lo + 1],
                            scalar2=None, op0=ALU.is_equal,
                        )
                        if first:
                            open_ps[w] = (
                                ps.tile([128, 512], dt.float32, tag="win", bufs=2, name="ps_win"),
                                ps.tile([128, 8], dt.float32, tag="small", name="ps_wd"),
                            )
                        ps_o, ps_d = open_ps[w]
                        nc.tensor.matmul(out=ps_o[:, 0:OC], lhsT=oh[:], rhs=xgs[:, 0:OC],
                                         start=first, stop=last)
                        nc.tensor.matmul(out=ps_d[:, 0:H], lhsT=oh[:],
                                         rhs=exb[:, tl * H : (tl + 1) * H],
                                         start=first, stop=last)
                        if last:
                            flush(w, open_ps.pop(w))

            assert not open_ps

        def make_pass1(l, ps_s, ps_q, exps):
            IN, OC, H, EW, AC = LCFG[l]
            rs_out = rsA if l < 2 else rsC
            nch = OC // 128

            def pass1_chunk(kc):
                for g5 in range(kc * RB // GRP, (kc + 1) * RB // GRP):
                    # batched loads: RS rows + own x rows for 5 blocks
                    ld5 = sb.tile([128, GRP * AC], dt.bfloat16, tag="ld5", bufs=2, name="ld5_t")
                    nc.sync.dma_start(
                        ld5[:].rearrange("p (t w) -> p t w", w=AC),
                        rs_out[g5 * GRP * 128 : (g5 + 1) * GRP * 128, :].rearrange("(t p) w -> p t w", p=128),
                    )
                    xl5 = sb.tile([128, GRP * OC], dt.bfloat16, tag="xl5", bufs=2, name="xl5_t")
                    nc.sync.dma_start(
                        xl5[:].rearrange("p (t w) -> p t w", w=OC),
                        x_tbl[g5 * GRP * 128 : (g5 + 1) * GRP * 128, 0:OC].rearrange("(t p) w -> p t w", p=128),
                    )
                    for j in range(GRP):
                        nt = g5 * GRP + j
                        blk = ld5[:, j * AC : (j + 1) * AC]
                        xblk = xl5[:, j * OC : (j + 1) * OC]
                        # denominator: den_rs + exp_self + eps -> masked reciprocal
                        d4 = sb.tile([128, 4], dt.float32, tag="d4", bufs=2)
                        nc.vector.tensor_scalar(out=d4[:, 0:H], in0=blk[:, OC : OC + H],
                                                scalar1=EPS_DEN, scalar2=None, op0=ALU.add)
                        nc.vector.tensor_tensor(out=d4[:, 0:H], in0=d4[:, 0:H],
                                                in1=exps[:, nt * 4 : nt * 4 + H], op=ALU.add)
                        r4 = sb.tile([128, 4], dt.float32, tag="r4", bufs=2)
                        nc.vector.reciprocal(r4[:, 0:H], d4[:, 0:H])
                        nc.vector.tensor_scalar(out=r4[:, 0:H], in0=r4[:, 0:H],
                                                scalar1=mask_sb[:, nt : nt + 1], scalar2=None, op0=ALU.mult)
                        # numerator: num_rs + exp_self * x_i, then divide
                        nsum = sb.tile([128, 512], dt.bfloat16, tag="nsum", bufs=2)
                        for h in range(H):
                            cs = slice(h * 128, (h + 1) * 128) if H > 1 else slice(0, OC)
                            nc.vector.tensor_scalar(
                                out=nsum[:, cs], in0=xblk[:, cs],
                                scalar1=exps[:, nt * 4 + h : nt * 4 + h + 1],
                                scalar2=None, op0=ALU.mult,
                            )
                        nc.vector.tensor_tensor(out=nsum[:, 0:OC], in0=nsum[:, 0:OC],
                                                in1=blk[:, 0:OC], op=ALU.add)
                        hblk = sb.tile([128, 512], dt.bfloat16, tag="hblk", bufs=2)
                        for h in range(H):
                            cs = slice(h * 128, (h + 1) * 128) if H > 1 else slice(0, OC)
                            nc.vector.tensor_scalar(
                                out=hblk[:, cs], in0=nsum[:, cs],
                                scalar1=r4[:, h : h + 1], scalar2=None, op0=ALU.mult,
                            )
                        sqb = sb.tile([128, 512], dt.bfloat16, tag="sqb", bufs=1)
                        nc.vector.tensor_tensor(out=sqb[:, 0:OC], in0=hblk[:, 0:OC], in1=hblk[:, 0:OC], op=ALU.mult)
                        nc.tensor.matmul(out=ps_s[:], lhsT=onesb_sb[:], rhs=hblk[:, 0:OC],
                                         start=(nt == 0), stop=(nt == NBLK - 1))
                        nc.tensor.matmul(out=ps_q[:], lhsT=onesb_sb[:], rhs=sqb[:, 0:OC],
                                         start=(nt == 0), stop=(nt == NBLK - 1))
                        for kk in range(nch):
                            ps_t = ps.tile([128, 128], dt.bfloat16, tag="med")
                            nc.tensor.matmul(out=ps_t[:], lhsT=hblk[:, kk * 128 : (kk + 1) * 128],
                                             rhs=idenb_sb[:], is_transpose=True, start=True, stop=True)
                            if (nt + kk) % 2 == 0:
                                nc.scalar.activation(hTe[:, kk * N2 + nt * 128 : kk * N2 + (nt + 1) * 128], ps_t[:], ACTF.Copy)
                            else:
                                nc.vector.tensor_copy(hTe[:, kk * N2 + nt * 128 : kk * N2 + (nt + 1) * 128], ps_t[:])

            return pass1_chunk

        def post_phase(l, ps_s, ps_q):
            IN, OC, H, EW, AC = LCFG[l]
            nch = OC // 128
            # BN stats AllReduce -> affine coeffs
            gam_t = sb.tile([1, OC], dt.float32, tag="gamt", bufs=1)
            nc.sync.dma_start(gam_t[:], P[f"gam{l}"][0:1, 0:OC])
            bet_t = sb.tile([1, OC], dt.float32, tag="bett", bufs=1)
            nc.sync.dma_start(bet_t[:], P[f"bet{l}"][0:1, 0:OC])
            stats = sb.tile([1, 2 * OC], dt.float32, tag="stats", bufs=1)
            nc.vector.tensor_copy(stats[0:1, 0:OC], ps_s)
            nc.vector.tensor_copy(stats[0:1, OC : 2 * OC], ps_q)
            st_in = dram.tile([1, 2 * OC], dt.float32, tag="st_in")
            st_out = dram.tile([1, 2 * OC], dt.float32, tag=f"st_out{l}", addr_space="Shared", name=f"st_out{l}")
            nc.sync.dma_start(st_in[:], stats[:])
            nc.gpsimd.collective_compute(
                "AllReduce", ALU.add, ins=[st_in[:]], outs=[st_out[:]], replica_groups=[list(range(NC))]
            )
            st2 = sb.tile([1, 2 * OC], dt.float32, tag="stats", bufs=1)
            nc.sync.dma_start(st2[:], st_out[:])
            m = sb.tile([1, OC], dt.float32, tag="bn_m", bufs=1)
            q = sb.tile([1, OC], dt.float32, tag="bn_q", bufs=1)
            nc.vector.tensor_scalar(out=m[:], in0=st2[0:1, 0:OC], scalar1=1.0 / N_NODES, scalar2=None, op0=ALU.mult)
            nc.vector.tensor_scalar(out=q[:], in0=st2[0:1, OC : 2 * OC], scalar1=1.0 / N_NODES, scalar2=None, op0=ALU.mult)
            var = sb.tile([1, OC], dt.float32, tag="bn_v", bufs=1)
            nc.vector.tensor_tensor(out=var[:], in0=m[:], in1=m[:], op=ALU.mult)
            nc.vector.tensor_tensor(out=var[:], in0=q[:], in1=var[:], op=ALU.subtract)
            epsc = sb.tile([1, 1], dt.float32, tag="epsc")
            nc.vector.memset(epsc[:], EPS)
            sd = sb.tile([1, OC], dt.float32, tag="bn_sd", bufs=1)
            nc.scalar.activation(sd[:], var[:], ACTF.Sqrt, bias=epsc[0:1, 0:1])
            rs = sb.tile([1, OC], dt.float32, tag="bn_rs", bufs=1)
            nc.vector.reciprocal(rs[:], sd[:])
            s_row = sb.tile([1, OC], dt.float32, tag="bn_s", bufs=1)
            nc.vector.tensor_tensor(out=s_row[:], in0=rs[:], in1=gam_t[:], op=ALU.mult)
            b_row = sb.tile([1, OC], dt.float32, tag="bn_b", bufs=1)
            nc.vector.tensor_tensor(out=b_row[:], in0=m[:], in1=s_row[:], op=ALU.mult)
            nc.vector.tensor_tensor(out=b_row[:], in0=bet_t[:], in1=b_row[:], op=ALU.subtract)
            sbc = sb.tile([128, 2 * nch], dt.float32, tag="sbc")
            for k in range(nch):
                ps_c = ps.tile([128, 1], dt.float32, tag="small")
                nc.tensor.matmul(out=ps_c[:], lhsT=s_row[0:1, k * 128 : (k + 1) * 128], rhs=onesr_sb[0:1, 0:1], start=True, stop=True)
                nc.vector.tensor_copy(sbc[:, k : k + 1], ps_c[:])
                ps_c2 = ps.tile([128, 1], dt.float32, tag="small")
                nc.tensor.matmul(out=ps_c2[:], lhsT=b_row[0:1, k * 128 : (k + 1) * 128], rhs=onesr_sb[0:1, 0:1], start=True, stop=True)
                nc.vector.tensor_copy(sbc[:, nch + k : nch + k + 1], ps_c2[:])
            # affine + ELU in place on hTe, stripes of 4 blocks
            SW = 4
            for s0 in range(0, NBLK, SW):
                sw = min(SW, NBLK - s0)
                W = sw * 128
                for k in range(nch):
                    ystr = sb.tile([128, SW * 128], dt.bfloat16, tag="ystr", bufs=1)
                    nc.vector.tensor_scalar(
                        out=ystr[:, 0:W], in0=hTe[:, k * N2 + s0 * 128 : k * N2 + s0 * 128 + W],
                        scalar1=sbc[:, k : k + 1], scalar2=sbc[:, nch + k : nch + k + 1],
                        op0=ALU.mult, op1=ALU.add,
                    )
                    t1 = sb.tile([128, SW * 128], dt.bfloat16, tag="elu1", bufs=1)
                    nc.vector.tensor_scalar(out=t1[:, 0:W], in0=ystr[:, 0:W], scalar1=0.0, scalar2=None, op0=ALU.min)
                    e1 = sb.tile([128, SW * 128], dt.bfloat16, tag="elu2", bufs=1)
                    nc.scalar.activation(e1[:, 0:W], t1[:, 0:W], ACTF.Exp)
                    r1 = sb.tile([128, SW * 128], dt.bfloat16, tag="elu3", bufs=1)
                    nc.vector.tensor_scalar(out=r1[:, 0:W], in0=ystr[:, 0:W], scalar1=0.0, scalar2=-1.0, op0=ALU.max, op1=ALU.add)
                    nc.vector.tensor_tensor(
                        out=hTe[:, k * N2 + s0 * 128 : k * N2 + s0 * 128 + W],
                        in0=e1[:, 0:W], in1=r1[:, 0:W], op=ALU.add,
                    )

        for l in range(3):
            exps = x_phase(l)
            ps_s = ps.tile([1, 512], dt.float32, tag="row", name="ps_s")
            ps_q = ps.tile([1, 512], dt.float32, tag="row", name="ps_q")
            ps_s = ps_s[:, 0 : LCFG[l][1]]
            ps_q = ps_q[:, 0 : LCFG[l][1]]
            p1 = make_pass1(l, ps_s, ps_q, exps)
            edge_phase(l)
            for kc in range(K_CH):
                p1(kc)
            post_phase(l, ps_s, ps_q)

        # =========================================================
        # pooling + classifier
        # =========================================================
        meanT = sb1.tile([128, N_GRAPHS], dt.float32, tag="meanT")
        maxT = sb1.tile([128, N_GRAPHS], dt.float32, tag="maxT")
        nc.vector.memset(meanT[:], 0)
        nc.vector.memset(maxT[:], -1e30)
        mask8_sb = load_sb(i_mask8, [128, NC], dt.float32, "mask8")
        mask8n_sb = load_sb(i_mask8n, [128, NC], dt.float32, "mask8n")
        for cc in range(NC):
            scrm = sb.tile([128, N_GRAPHS], dt.float32, tag="scrm", bufs=1)
            scrx = sb.tile([128, N_GRAPHS], dt.float32, tag="scrx", bufs=1)
            nc.vector.memset(scrm[:], 0)
            nc.vector.memset(scrx[:], -1e30)
            for (a, bnd, g, inv) in segs[cc]:
                r1 = sb.tile([128, 1], dt.float32, tag="segr")
                nc.vector.tensor_reduce(out=r1[:], in_=hTe[:, a:bnd], axis=AXX, op=ALU.add)
                nc.vector.tensor_scalar(out=scrm[:, g : g + 1], in0=r1[:], scalar1=inv, scalar2=None, op0=ALU.mult)
                nc.vector.tensor_reduce(out=scrx[:, g : g + 1], in_=hTe[:, a:bnd], axis=AXX, op=ALU.max)
            nc.vector.tensor_scalar(out=scrm[:], in0=scrm[:], scalar1=mask8_sb[:, cc : cc + 1], scalar2=None, op0=ALU.mult)
            nc.vector.tensor_tensor(out=meanT[:], in0=meanT[:], in1=scrm[:], op=ALU.add)
            nc.vector.tensor_scalar(
                out=scrx[:], in0=scrx[:], scalar1=mask8_sb[:, cc : cc + 1],
                scalar2=mask8n_sb[:, cc : cc + 1], op0=ALU.mult, op1=ALU.add,
            )
            nc.vector.tensor_tensor(out=maxT[:], in0=maxT[:], in1=scrx[:], op=ALU.max)
        pm_in = dram.tile([128, N_GRAPHS], dt.float32, tag="pm_in")
        pm_out = dram.tile([128, N_GRAPHS], dt.float32, tag="pm_out", addr_space="Shared")
        px_in = dram.tile([128, N_GRAPHS], dt.float32, tag="px_in")
        px_out = dram.tile([128, N_GRAPHS], dt.float32, tag="px_out", addr_space="Shared")
        nc.sync.dma_start(pm_in[:], meanT[:])
        nc.sync.dma_start(px_in[:], maxT[:])
        nc.gpsimd.collective_compute("AllReduce", ALU.add, ins=[pm_in[:]], outs=[pm_out[:]], replica_groups=[list(range(NC))])
        nc.gpsimd.collective_compute("AllReduce", ALU.max, ins=[px_in[:]], outs=[px_out[:]], replica_groups=[list(range(NC))])
        meanF, maxF = meanT, maxT
        nc.sync.dma_start(meanF[:], pm_out[:])
        nc.sync.dma_start(maxF[:], px_out[:])

        cw1t_sb = [None, None]
        cw2t_sb = [None, None]
        for k in range(2):
            cw1t_sb[k] = sb1.tile([128, GDIM], dt.float32, tag=f"cw1t{k}", name=f"cw1t{k}")
            nc.sync.dma_start(cw1t_sb[k][:], P["cw1t"][k * 128 : (k + 1) * 128, :])
            cw2t_sb[k] = sb1.tile([128, NUM_CLASSES], dt.float32, tag=f"cw2t{k}", name=f"cw2t{k}")
            nc.sync.dma_start(cw2t_sb[k][:], P["cw2t"][k * 128 : (k + 1) * 128, :])
        cb1c_sb = load_sb(P["cb1c"], [128, 2], dt.float32, "cb1c")
        cb2c_sb = load_sb(P["cb2c"], [NUM_CLASSES, 1], dt.float32, "cb2c")
        hidT = sb1.tile([128, 2 * N_GRAPHS], dt.float32, tag="hidT")
        for hc in range(2):
            ps_h = ps.tile([128, N_GRAPHS], dt.float32, tag="med", name="ps_h")
            for dc, embT in enumerate([meanF, maxF]):
                nc.tensor.matmul(
                    out=ps_h[:], lhsT=cw1t_sb[dc][:, hc * 128 : (hc + 1) * 128],
                    rhs=embT[:], start=(dc == 0), stop=(dc == 1),
                )
            nc.vector.tensor_scalar(
                out=hidT[:, hc * N_GRAPHS : (hc + 1) * N_GRAPHS], in0=ps_h[:],
                scalar1=cb1c_sb[:, hc : hc + 1], scalar2=0.0, op0=ALU.add, op1=ALU.max,
            )
        ps_o = ps.tile([NUM_CLASSES, N_GRAPHS], dt.float32, tag="med", name="ps_out")
        for hc in range(2):
            nc.tensor.matmul(
                out=ps_o[:], lhsT=cw2t_sb[hc][:],
                rhs=hidT[:, hc * N_GRAPHS : (hc + 1) * N_GRAPHS], start=(hc == 0), stop=(hc == 1),
            )
        osb = sb1.tile([NUM_CLASSES, N_GRAPHS], dt.float32, tag="osb")
        nc.vector.tensor_scalar(out=osb[:], in0=ps_o[:], scalar1=cb2c_sb[:], scalar2=None, op0=ALU.add)
        for gc in range(2):
            ps_tt = ps.tile([128, NUM_CLASSES], dt.float32, tag="med", name="ps_tt")
            nc.tensor.matmul(
                out=ps_tt[:], lhsT=osb[:, gc * 128 : (gc + 1) * 128],
                rhs=idenf_sb[0:NUM_CLASSES, 0:NUM_CLASSES], start=True, stop=True,
            )
            ot = sb1.tile([128, NUM_CLASSES], dt.float32, tag="ot")
            nc.vector.tensor_copy(ot[:], ps_tt[:])
            nc.sync.dma_start(out_dram[gc * 128 : (gc + 1) * 128, :], ot[:])

    nc.compile()
    return nc


def kernel(**inputs):
    import concourse.bass_utils as bass_utils
    import hashlib

    pre = preprocess(inputs["x"], inputs["edge_index"], inputs["depth"], inputs["batch"])
    pb = build_param_blobs(inputs)

    sch = pre["sched"]
    sig = hashlib.sha1(
        sch["slots_w"].tobytes() + repr(sch["spans"]).encode() + repr(sch["segs"]).encode()
    ).hexdigest()
    if _CACHE.get("sig") != sig:
        _CACHE["built"] = build_nc(pre)
        _CACHE["sig"] = sig
    nc = _CACHE["built"]

    in_maps = []
    for c in range(NC):
        b = pre["blobs"][c]
        m = dict(
            xidx=b["xidx"], aldidx=b["aldidx"], dstloc=b["dstloc"],
            emb_idx=b["emb_idx"], depth_row=b["depth_row"],
            mask8=b["mask8"], mask8n=b["mask8n"], maskcol=b["maskcol"],
        )
        m.update(pb)
        in_maps.append(m)

    import os, time

    trace = bool(int(os.environ.get("KERNEL_TRACE", "0")))
    t0 = time.time()
    res = bass_utils.run_bass_kernel_spmd(
        nc, in_maps, core_ids=list(range(NC)), trace=trace
    )
    _CACHE["run_s"] = time.time() - t0
    _CACHE["last_results"] = res
    return np.asarray(res.results[0]["out"], dtype=np.float32)


if __name__ == "__main__":
    sys.path.insert(0, "/root/problem")
    import reference

    inp = {k: np.asarray(v) for k, v in reference.setup_inputs().items()}
    got = kernel(**inp)
    exp = np.asarray(reference.reference(**inp))
    err = np.abs(got - exp).max() / (np.abs(exp).max() + 1e-30)
    print("Relative error:", err)


# revision 41
# speedup vs baseline: 1.1270x; 1.1270x over previous
"""Trainium2 Bass kernel for nn_ASTGATClassifier (3-layer GAT + BN + ELU + pool + MLP).

v3 strategy (8 NeuronCores, SPMD single program), built around the TimelineSim
cost model's pricing (DMA per-descriptor, collectives 15us const + out bytes):

  - Edges SRC-partitioned; GAT softmax division deferred past a bf16
    ReduceScatter of a [NTOT, OC+H] (numerator ++ denominator) accumulator.
  - Self-loops are REMOVED from the edge stream (they concentrated 128
    slots/window on the dst-owner core, inflating the uniform per-window
    slot budget by ~55%) and folded in algebraically after the RS:
    out = (num_rs + exp_self*x_i) / (den_rs + exp_self), with
    exp_self = exp(leaky(als_i + ald_i)) computed core-locally.
  - K_CH=2 RS chunks (15us constant per collective), NBLK=50 (N2=6400).
  - Per-window slot counts are the exact max over cores (no ceil16);
    (chunk, ald-half) runs still pad to 128.
  - Scatter-add via one-hot matmuls into per-128-dst-window PSUM; windows
    flush through a 5-window bf16 stage; region DMAs feed the chunk RS.
  - al_d travels via a tiny AllGather of [N2,4] + DRAM expansion into 256B
    rows for the per-edge gather.  BN stats via ones-matmuls + AllReduce,
    with pad rows masked.  Pooling + classifier as before.
"""

import sys

sys.path.insert(0, "/opt/trn_rl_repo")

import numpy as np
import ml_dtypes

N_NODES = 50000
N_EDGES = 400000
N_GRAPHS = 256
NUM_TYPES = 200
EMB = 64
HID = 128
HEADS = 4
GDIM = 256
NUM_CLASSES = 20
EPS = 1e-5
NEG = 0.2
EPS_DEN = 1e-20

NC = 8
NSH = N_NODES // NC          # 6250 nodes per core
NBLK = 50                    # node blocks per core (50*128 = 6400)
N2 = NBLK * 128              # padded shard
NTOT = NC * N2               # 51200
K_CH = 2                     # RS chunks
CH = N2 // K_CH              # 3200 rows per (chunk, core) region
RB = CH // 128               # 25 windows per region
NW = K_CH * NC * RB          # 400 windows
ALD_HALF = (NC // 2) * N2    # 25600
GRP = 5                      # windows per stage/flush group

# per-layer config: (IN, OC, H, EW(gather row cols), AC(accum cols),
#                    VW(value cols per head), PERHEAD(row has per-head values))
# L0 aggregates exp*h (64-wide, shared across heads) and transforms by W0
# after the ReduceScatter: sum_e exp*(h@W) == (sum_e exp*h)@W per head.
LCFG = [
    (EMB, 512, 4, 128, 260, EMB, False),
    (512, 512, 4, 640, 516, 128, True),
    (512, 128, 1, 256, 129, 128, True),
]
SPAN_SLOTS = 1792            # max slots per gather call (14 tiles)

BF16 = ml_dtypes.bfloat16

_CACHE = {}


def _wrap_idx(idx):
    """int16 gather index layout: [128, n/16]; idx j at [j%16, j//16], tiled x8."""
    n = len(idx)
    assert n % 16 == 0
    a = np.asarray(idx, dtype=np.int16).reshape(n // 16, 16).T
    return np.tile(a, (8, 1))


def preprocess(x, edge_index, depth, batch):
    """Host-side index preprocessing -> per-core blobs + uniform schedule."""
    x = np.asarray(x).astype(np.int64)
    ei = np.asarray(edge_index).astype(np.int64)
    batch = np.asarray(batch).astype(np.int64)
    src = ei[0]
    dst = ei[1]

    # destination-side row mappings (global)
    oc = dst // NSH
    locd = dst - oc * NSH
    kch = locd // CH
    arow = kch * (NC * CH) + oc * CH + (locd - kch * CH)   # accum row (chunk-major)
    wind = arow // 128
    d128 = arow % 128
    aldrow = oc * N2 + locd                                 # ald table row
    half = (oc >= NC // 2).astype(np.int64)
    aldidx = aldrow - half * ALD_HALF

    core_of_src = src // NSH
    percore = []
    cnts = np.zeros((NC, NW), dtype=np.int64)
    for c in range(NC):
        m = core_of_src == c
        sl = (src[m] - c * NSH).astype(np.int64)
        wc, ac, dc, aic = wind[m], arow[m], d128[m], aldidx[m]
        order = np.argsort(ac, kind="stable")
        sl, wc, dc, aic = sl[order], wc[order], dc[order], aic[order]
        percore.append((sl, wc, dc, aic))
        cnts[c] = np.bincount(wc, minlength=NW)

    slots_w = cnts.max(axis=0).astype(np.int64)   # exact max, no ceil

    # build slot stream: windows in order; pad each (k, half) run to %128
    slot_w = []          # per-slot window id (-1 = run pad)
    ws_start = np.zeros(NW, dtype=np.int64)
    run_bounds = []      # (slot_lo, slot_hi, half) per run
    pos = 0
    for k in range(K_CH):
        for hf in range(2):
            run_lo = pos
            w0 = k * NC * RB + hf * (NC // 2) * RB
            for w in range(w0, w0 + (NC // 2) * RB):
                ws_start[w] = pos
                slot_w.extend([w] * int(slots_w[w]))
                pos += int(slots_w[w])
            r = (-pos) % 128
            slot_w.extend([-1] * r)
            pos += r
            run_bounds.append((run_lo, pos, hf))
    S = pos
    assert S % 128 == 0
    slot_w = np.asarray(slot_w, dtype=np.int64)
    n_tiles = S // 128

    # spans: cut runs into <= SPAN_SLOTS pieces (128-aligned)
    spans = []
    for (lo, hi, hf) in run_bounds:
        p = lo
        while p < hi:
            n = min(SPAN_SLOTS, hi - p)
            spans.append((p, n, hf))
            p += n

    # pairs: per tile, windows overlapping its slot range
    ws_end = ws_start + slots_w
    pairs = []            # (tile, w, first, last)
    pairs_of_tile = [[] for _ in range(n_tiles)]
    for w in range(NW):
        if slots_w[w] == 0:
            continue
        t0 = int(ws_start[w] // 128)
        t1 = int((ws_end[w] - 1) // 128)
        for t in range(t0, t1 + 1):
            pc = len(pairs)
            pairs.append([t, w, t == t0, t == t1])
            pairs_of_tile[t].append(pc)
    NPAIR = len(pairs)

    # per-span contiguous pair-column range (pairs are (w, t)-lex ordered,
    # which matches slot order, so each span's pairs are contiguous)
    span_pc = []
    for (s0, ns, hf) in spans:
        pcs = []
        for t in range(s0 // 128, (s0 + ns) // 128):
            pcs.extend(pairs_of_tile[t])
        assert pcs == list(range(pcs[0], pcs[-1] + 1))
        assert len(pcs) <= 4 * (SPAN_SLOTS // 128)
        span_pc.append((pcs[0], pcs[-1] + 1))

    # per (region, group): the last non-empty window triggers the group flush
    # group id = (k, oc, g);  windows w0+g*GRP .. w0+(g+1)*GRP-1
    flush_trigger = {}    # w -> list of (k, oc, g) groups it closes
    for k in range(K_CH):
        for occ in range(NC):
            r0 = (k * NC + occ) * RB
            for g in range(RB // GRP):
                ws = [r0 + g * GRP + j for j in range(GRP)]
                nonempty = [w for w in ws if slots_w[w] > 0]
                assert nonempty, "fully-empty flush group"
                flush_trigger.setdefault(nonempty[-1], []).append((k, occ, g))

    # per-core blobs
    blobs = []
    for c in range(NC):
        sl, wc, dc, aic = percore[c]
        xidx = np.zeros(S, dtype=np.int64)
        aidx = np.zeros(S, dtype=np.int64)
        dloc = np.full(S, -1.0, dtype=np.float32)
        offs = ws_start[wc]
        within = np.zeros(len(wc), dtype=np.int64)
        uw, uidx, ucnt = np.unique(wc, return_index=True, return_counts=True)
        for u, i0, cc in zip(uw, uidx, ucnt):
            within[i0 : i0 + cc] = np.arange(cc)
        slot = offs + within
        xidx[slot] = sl
        aidx[slot] = aic
        dloc[slot] = dc.astype(np.float32)
        # dstloc blob per pair
        dl = np.full((128, NPAIR), -1.0, dtype=np.float32)
        for pc, (t, w, fi, la) in enumerate(pairs):
            seg = dloc[t * 128 : (t + 1) * 128].copy()
            m = slot_w[t * 128 : (t + 1) * 128] != w
            seg[m] = -1.0
            dl[:, pc] = seg
        # per-span interleaved [xi | ai] index stream -> one DMA per span
        xiw = _wrap_idx(xidx)
        aiw = _wrap_idx(aidx)
        cols = []
        for (s0, ns, hf) in spans:
            cols.append(xiw[:, s0 // 16 : (s0 + ns) // 16])
            cols.append(aiw[:, s0 // 16 : (s0 + ns) // 16])
        blobs.append(
            dict(
                eidx=np.ascontiguousarray(np.concatenate(cols, axis=1)),
                dstloc=dl,
            )
        )

    # emb gather idx + depth rows per core (layer-0 prolog) + pad mask
    mask = np.zeros((128, NBLK), dtype=np.float32)
    for nt in range(NBLK):
        mask[:, nt] = (nt * 128 + np.arange(128)) < NSH
    for c in range(NC):
        ids = np.zeros(N2, dtype=np.int64)
        ids[:NSH] = x[c * NSH : (c + 1) * NSH]
        blobs[c]["emb_idx"] = _wrap_idx(ids)
        dc_ = np.zeros(N2, dtype=np.float32)
        dc_[:NSH] = np.asarray(depth, dtype=np.float32)[c * NSH : (c + 1) * NSH]
        blobs[c]["depthc"] = np.ascontiguousarray(dc_.reshape(NBLK, 128).T)  # [128, NBLK]
        blobs[c]["maskcol"] = mask

    # pooling segments
    counts = np.bincount(batch, minlength=N_GRAPHS)
    starts = np.concatenate([[0], np.cumsum(counts)])
    segs = []
    for cc in range(NC):
        lo_n, hi_n = cc * NSH, (cc + 1) * NSH
        lst = []
        for g in range(N_GRAPHS):
            a, bnd = starts[g], starts[g + 1]
            aa, bb = max(a, lo_n), min(bnd, hi_n)
            if aa < bb:
                lst.append((int(aa - lo_n), int(bb - lo_n), int(g), float(1.0 / max(counts[g], 1))))
        segs.append(lst)
    for c in range(NC):
        m8 = np.zeros((128, NC), dtype=np.float32)
        m8[:, c] = 1.0
        m8n = np.where(m8 > 0, 0.0, -1e30).astype(np.float32)
        blobs[c]["mask8"] = m8
        blobs[c]["mask8n"] = m8n

    sched = dict(
        slots_w=slots_w, ws_start=ws_start, ws_end=ws_end, spans=spans,
        pairs=pairs, pairs_of_tile=pairs_of_tile, n_tiles=n_tiles, S=S,
        NPAIR=NPAIR, segs=segs, flush_trigger=flush_trigger, span_pc=span_pc,
    )
    return dict(sched=sched, blobs=blobs)


def build_param_blobs(p):
    """Host-side parameter layout transforms (bf16 casts, folds, transposes)."""
    f32 = np.float32
    out = {}

    def fold_a(W, a_s, a_d, heads, c):
        W3 = W.reshape(heads, c, -1)
        As = np.einsum("hck,hc->kh", W3, a_s).astype(f32)
        Ad = np.einsum("hck,hc->kh", W3, a_d).astype(f32)
        return np.concatenate([As, Ad], axis=1)  # [IN, 2H]

    out["w0x"] = np.ascontiguousarray(p["W0"].T).astype(BF16)
    out["w0al"] = fold_a(p["W0"], p["as0"], p["ad0"], HEADS, HID).astype(BF16)
    out["w1x"] = np.ascontiguousarray(p["W1"].T).astype(BF16)
    out["w1al"] = fold_a(p["W1"], p["as1"], p["ad1"], HEADS, HID).astype(BF16)
    out["w2x"] = np.ascontiguousarray(p["W2"].T).astype(BF16)
    out["w2al"] = fold_a(p["W2"], p["as2"], p["ad2"], 1, GDIM // 2).astype(BF16)
    # depth bias folded into the embedding table (h = emb[x] + depth*dw + db)
    out["emb_t"] = np.asarray(p["emb_table"], dtype=f32) + np.asarray(p["depth_b"], dtype=f32).reshape(1, EMB)
    out["dwb"] = np.tile(np.asarray(p["depth_w"], dtype=f32).reshape(1, EMB), (128, 1))
    for l, (g, be) in enumerate([(p["g0"], p["be0"]), (p["g1"], p["be1"]), (p["g2"], p["be2"])]):
        out[f"gam{l}"] = np.asarray(g, dtype=f32).reshape(1, -1)
        out[f"bet{l}"] = np.asarray(be, dtype=f32).reshape(1, -1)
    out["cw1t"] = np.ascontiguousarray(p["cw1"].T).astype(f32)
    out["cb1c"] = np.asarray(p["cb1"], dtype=f32).reshape(2, 128).T.copy()
    out["cw2t"] = np.ascontiguousarray(p["cw2"].T).astype(f32)
    out["cb2c"] = np.asarray(p["cb2"], dtype=f32).reshape(NUM_CLASSES, 1)
    out["iotab"] = np.tile(np.arange(128, dtype=f32)[None, :], (128, 1)).astype(BF16)
    out["iden_f"] = np.eye(128, dtype=f32)
    out["iden_b"] = np.eye(128).astype(BF16)
    out["ones_b"] = np.ones((128, 1), dtype=BF16)
    out["ones_r"] = np.ones((1, 128), dtype=f32)
    return out


def build_nc(pre):
    """Trace the full SPMD bass program (structure from `pre['sched']`)."""
    import concourse.bacc as bacc
    import concourse.bass as bass
    import concourse.mybir as mybir
    import concourse.tile as tile
    from concourse.library_config import mlp
    from contextlib import ExitStack

    dt = mybir.dt
    ALU = mybir.AluOpType
    ACTF = mybir.ActivationFunctionType
    AXX = mybir.AxisListType.X

    sch = pre["sched"]
    spans = sch["spans"]
    pairs = sch["pairs"]
    pairs_of_tile = sch["pairs_of_tile"]
    n_tiles = sch["n_tiles"]
    S = sch["S"]
    NPAIR = sch["NPAIR"]
    slots_w = sch["slots_w"]
    segs = sch["segs"]
    flush_trigger = sch["flush_trigger"]
    span_pc = sch["span_pc"]

    nc = bacc.Bacc("TRN2", target_bir_lowering=False, debug=False, num_devices=NC)

    b0 = pre["blobs"][0]
    EIN = {}

    def ein(name, arr_like, dtyp):
        EIN[name] = nc.dram_tensor(name, list(arr_like.shape), dtyp, kind="ExternalInput").ap()
        return EIN[name]

    i_eidx = ein("eidx", b0["eidx"], dt.int16)
    i_dstloc = ein("dstloc", b0["dstloc"], dt.float32)
    i_embidx = ein("emb_idx", b0["emb_idx"], dt.int16)
    i_depthc = ein("depthc", b0["depthc"], dt.float32)
    i_mask8 = ein("mask8", b0["mask8"], dt.float32)
    i_mask8n = ein("mask8n", b0["mask8n"], dt.float32)
    i_maskcol = ein("maskcol", b0["maskcol"], dt.float32)
    P = {}
    P["w0x"] = ein("w0x", np.zeros((EMB, 512)), dt.bfloat16)
    P["w0al"] = ein("w0al", np.zeros((EMB, 8)), dt.bfloat16)
    P["w1x"] = ein("w1x", np.zeros((512, 512)), dt.bfloat16)
    P["w1al"] = ein("w1al", np.zeros((512, 8)), dt.bfloat16)
    P["w2x"] = ein("w2x", np.zeros((512, 128)), dt.bfloat16)
    P["w2al"] = ein("w2al", np.zeros((512, 2)), dt.bfloat16)
    P["emb_t"] = ein("emb_t", np.zeros((NUM_TYPES, EMB)), dt.float32)
    P["dwb"] = ein("dwb", np.zeros((128, EMB)), dt.float32)
    for l, ocl in [(0, 512), (1, 512), (2, 128)]:
        P[f"gam{l}"] = ein(f"gam{l}", np.zeros((1, ocl)), dt.float32)
        P[f"bet{l}"] = ein(f"bet{l}", np.zeros((1, ocl)), dt.float32)
    P["cw1t"] = ein("cw1t", np.zeros((GDIM, GDIM)), dt.float32)
    P["cb1c"] = ein("cb1c", np.zeros((128, 2)), dt.float32)
    P["cw2t"] = ein("cw2t", np.zeros((GDIM, NUM_CLASSES)), dt.float32)
    P["cb2c"] = ein("cb2c", np.zeros((NUM_CLASSES, 1)), dt.float32)
    P["iotab"] = ein("iotab", np.zeros((128, 128)), dt.bfloat16)
    P["iden_f"] = ein("iden_f", np.zeros((128, 128)), dt.float32)
    P["iden_b"] = ein("iden_b", np.zeros((128, 128)), dt.bfloat16)
    P["ones_b"] = ein("ones_b", np.zeros((128, 1)), dt.bfloat16)
    P["ones_r"] = ein("ones_r", np.zeros((1, 128)), dt.float32)

    out_dram = nc.dram_tensor("out", [N_GRAPHS, NUM_CLASSES], dt.float32, kind="ExternalOutput").ap()

    with tile.TileContext(nc) as tc, ExitStack() as stk:
        nc.gpsimd.load_library(mlp)
        sb = stk.enter_context(tc.tile_pool(name="sb", bufs=2))
        sb1 = stk.enter_context(tc.tile_pool(name="sb1", bufs=1))
        ps = stk.enter_context(tc.tile_pool(name="ps", bufs=2, space="PSUM"))
        dram = stk.enter_context(tc.tile_pool(name="dram", bufs=1, space="DRAM"))

        def load_sb(ap, shape, dtyp, tag, pool=sb1):
            t = pool.tile(shape, dtyp, tag=tag)
            nc.sync.dma_start(t[:], ap[:, :])
            return t

        iota_sb = load_sb(P["iotab"], [128, 128], dt.bfloat16, "iota")
        idenb_sb = load_sb(P["iden_b"], [128, 128], dt.bfloat16, "idenb")
        idenf_sb = load_sb(P["iden_f"], [128, 128], dt.float32, "idenf")
        onesb_sb = load_sb(P["ones_b"], [128, 1], dt.bfloat16, "onesb")
        onesr_sb = load_sb(P["ones_r"], [1, 128], dt.float32, "onesr")
        dwb_sb = load_sb(P["dwb"], [128, EMB], dt.float32, "dwb")
        depc_sb = load_sb(i_depthc, [128, NBLK], dt.float32, "depthc")
        mask_sb = load_sb(i_maskcol, [128, NBLK], dt.float32, "maskcol")
        dstloc_sb = load_sb(i_dstloc, [128, NPAIR], dt.float32, "dstloc")

        wx_sb = {}
        wal_sb = {}
        for l, (IN, OC, H, EW, AC, VW, PERHEAD) in enumerate(LCFG):
            nch_in = (IN + 127) // 128
            wx_sb[l] = []
            wal_sb[l] = []
            wxn, waln = f"w{l}x", f"w{l}al"
            for k in range(nch_in):
                kp = min(IN - k * 128, 128)
                tx = sb1.tile([kp, OC], dt.bfloat16, tag=f"wx{l}_{k}")
                nc.sync.dma_start(tx[:], P[wxn][k * 128 : k * 128 + kp, :])
                wx_sb[l].append(tx)
                ta = sb1.tile([kp, 2 * H], dt.bfloat16, tag=f"wal{l}_{k}")
                nc.sync.dma_start(ta[:], P[waln][k * 128 : k * 128 + kp, :])
                wal_sb[l].append(ta)

        # persistent transposed features (lhsT for x_phase; final = pooling input)
        hTe = sb1.tile([128, 4 * N2], dt.bfloat16, tag="hTe")

        # DRAM tensors
        x_tbl = dram.tile([N2, 640], dt.bfloat16, tag="x_tbl")
        h_tbl = dram.tile([N2, 128], dt.bfloat16, tag="h_tbl")
        ald_in = dram.tile([N2, 4], dt.bfloat16, tag="ald_in")
        ald_fulls = [
            dram.tile([NTOT, 4], dt.bfloat16, tag=f"ald_full{l}", addr_space="Shared", name=f"ald_full{l}")
            for l in range(3)
        ]
        ald_tbl = dram.tile([NTOT, 128], dt.bfloat16, tag="ald_tbl")
        accA = dram.tile([NTOT, 516], dt.bfloat16, tag="accA")
        accB = dram.tile([NTOT, 260], dt.bfloat16, tag="accB")
        accC = dram.tile([NTOT, 129], dt.bfloat16, tag="accC")
        rsA = dram.tile([N2, 516], dt.bfloat16, tag="rsA")
        rsB = dram.tile([N2, 260], dt.bfloat16, tag="rsB")
        rsC = dram.tile([N2, 129], dt.bfloat16, tag="rsC")
        ACCS = [(accB, rsB), (accA, rsA), (accC, rsC)]

        # =========================================================
        # Layer-0 prolog: hTe[0:64] = (emb + depth-proj)^T
        # =========================================================
        embidx_sb = load_sb(i_embidx, [128, N2 // 16], dt.int16, "embidx")
        emb_g = sb.tile([128, NBLK * EMB], dt.float32, tag="embg", bufs=1, name="emb_g")
        nc.gpsimd.dma_gather(
            emb_g[:].rearrange("p (t w) -> p t w", w=EMB),
            P["emb_t"][:, :],
            embidx_sb[:],
            N2, N2, EMB, single_packet=False,
        )
        for nt in range(NBLK):
            dep_t = sb.tile([128, EMB], dt.float32, tag="dep", bufs=2)
            nc.vector.tensor_scalar(out=dep_t[:], in0=dwb_sb[:], scalar1=depc_sb[:, nt : nt + 1],
                                    scalar2=None, op0=ALU.mult)
            nc.vector.tensor_tensor(
                out=emb_g[:, nt * EMB : (nt + 1) * EMB],
                in0=emb_g[:, nt * EMB : (nt + 1) * EMB], in1=dep_t[:], op=ALU.add,
            )
            ps_t = ps.tile([EMB, 128], dt.float32, tag="med")
            nc.tensor.matmul(
                out=ps_t[:],
                lhsT=emb_g[:, nt * EMB : (nt + 1) * EMB],
                rhs=idenf_sb[:],
                is_transpose=True,
                start=True,
                stop=True,
            )
            nc.vector.tensor_copy(hTe[0:EMB, nt * 128 : (nt + 1) * 128], ps_t[:])

        # =========================================================
        # per-layer phases
        # =========================================================
        def x_phase(l):
            IN, OC, H, EW, AC, VW, PERHEAD = LCFG[l]
            nch = (IN + 127) // 128

            def lhs(k, nt):
                kp = min(IN - k * 128, 128)
                return hTe[0:kp, k * N2 + nt * 128 : k * N2 + (nt + 1) * 128]

            # pass A: attention rows (al_s, al_d kept in SBUF; al_d staged + AllGather)
            als_sb = sb.tile([128, NBLK * 4], dt.float32, tag="als", bufs=1)
            alds = sb.tile([128, NBLK * 4], dt.bfloat16, tag="alds", bufs=1)
            nc.vector.memset(als_sb[:], 0)
            nc.vector.memset(alds[:], 0)
            for nt in range(NBLK):
                ps_al = ps.tile([128, 8], dt.float32, tag="small")
                for k in range(nch):
                    nc.tensor.matmul(out=ps_al[:, 0 : 2 * H], lhsT=lhs(k, nt), rhs=wal_sb[l][k][:],
                                     start=(k == 0), stop=(k == nch - 1))
                nc.vector.tensor_copy(als_sb[:, nt * 4 : nt * 4 + H], ps_al[:, 0:H])
                nc.vector.tensor_copy(alds[:, nt * 4 : nt * 4 + H], ps_al[:, H : 2 * H])
            nc.sync.dma_start(
                ald_in[:, :].rearrange("(t p) w -> p t w", p=128), alds[:].rearrange("p (t w) -> p t w", w=4)
            )
            nc.gpsimd.collective_compute(
                "AllGather", ALU.bypass, ins=[ald_in[:, :]], outs=[ald_fulls[l][:, :]],
                replica_groups=[list(range(NC))],
            )
            nc.sync.dma_start(ald_tbl[:, 0:4], ald_fulls[l][:, :])

            # self-loop exp: exps = exp(leaky(als_i + ald_i))  [128, NBLK*4] f32
            exps = sb.tile([128, NBLK * 4], dt.float32, tag="exps", bufs=1)
            nc.vector.tensor_tensor(out=exps[:], in0=als_sb[:], in1=alds[:], op=ALU.add)
            zneg = sb.tile([128, NBLK * 4], dt.float32, tag="zneg", bufs=1)
            nc.vector.tensor_scalar(out=zneg[:], in0=exps[:], scalar1=NEG, scalar2=None, op0=ALU.mult)
            nc.vector.tensor_tensor(out=exps[:], in0=exps[:], in1=zneg[:], op=ALU.max)
            nc.scalar.activation(exps[:], exps[:], ACTF.Exp)

            # pass B: value rows -> gather table (overlaps AllGather)
            if l == 0:
                # L0 gathers raw h (64) ++ als: write h_tbl rows from emb_g
                for g5 in range(NBLK // GRP):
                    xa5 = sb.tile([128, GRP * 68], dt.bfloat16, tag="xa5", bufs=2)
                    for j in range(GRP):
                        nt = g5 * GRP + j
                        nc.vector.tensor_copy(xa5[:, j * 68 : j * 68 + EMB], emb_g[:, nt * EMB : (nt + 1) * EMB])
                        nc.vector.tensor_copy(xa5[:, j * 68 + EMB : (j + 1) * 68], als_sb[:, nt * 4 : nt * 4 + H])
                    nc.sync.dma_start(
                        h_tbl[g5 * GRP * 128 : (g5 + 1) * GRP * 128, 0:68].rearrange("(t p) w -> p t w", p=128),
                        xa5[:].rearrange("p (t w) -> p t w", w=68),
                    )
            else:
                for nt in range(NBLK):
                    ps_x = ps.tile([128, 512], dt.float32, tag="win", bufs=2)
                    for k in range(nch):
                        nc.tensor.matmul(out=ps_x[:, 0:OC], lhsT=lhs(k, nt), rhs=wx_sb[l][k][:],
                                         start=(k == 0), stop=(k == nch - 1))
                    xa_t = sb.tile([128, EW], dt.bfloat16, tag="xa_t", bufs=2)
                    nc.scalar.activation(xa_t[:, 0:OC], ps_x[:, 0:OC], ACTF.Copy)
                    nc.vector.tensor_copy(xa_t[:, OC : OC + H], als_sb[:, nt * 4 : nt * 4 + H])
                    nc.sync.dma_start(x_tbl[nt * 128 : (nt + 1) * 128, 0:EW], xa_t[:])
            return exps

        def edge_phase(l):
            IN, OC, H, EW, AC, VW, PERHEAD = LCFG[l]
            acc, rs_out = ACCS[l]
            SCW = H * VW
            ALOFF = H * VW if PERHEAD else VW
            ST = SPAN_SLOTS // 128
            open_ps = {}
            stage = {"t": None}

            def flush(w, psw):
                ps_o, ps_d = psw
                r, b = w // RB, w % RB
                g, bj = b // GRP, b % GRP
                if stage["t"] is None:
                    stage["t"] = sb.tile([128, GRP * AC], dt.bfloat16, tag="stage", bufs=2, name="stage_t")
                    # zero slots of empty windows in this group (they never flush)
                    r0 = r * RB + g * GRP
                    for j in range(GRP):
                        if slots_w[r0 + j] == 0:
                            nc.vector.memset(stage["t"][:, j * AC : (j + 1) * AC], 0)
                dstc = stage["t"][:, bj * AC : bj * AC + SCW]
                nc.scalar.activation(dstc, ps_o[:, 0:SCW], ACTF.Copy)
                nc.vector.tensor_copy(stage["t"][:, bj * AC + SCW : bj * AC + SCW + H], ps_d[:, 0:H])
                for (k, occ, gg) in flush_trigger.get(w, ()):
                    rows0 = k * (NC * CH) + occ * CH + gg * GRP * 128
                    nc.sync.dma_start(
                        acc[rows0 : rows0 + GRP * 128, :].rearrange("(t p) w -> p t w", p=128),
                        stage["t"][:].rearrange("p (t w) -> p t w", w=AC),
                    )
                    stage["t"] = None
                    if occ == NC - 1 and gg == RB // GRP - 1:
                        nc.gpsimd.collective_compute(
                            "ReduceScatter", ALU.add,
                            ins=[acc[k * NC * CH : (k + 1) * NC * CH, :]],
                            outs=[rs_out[k * CH : (k + 1) * CH, :]],
                            replica_groups=[list(range(NC))],
                        )

            for si, (s0, ns, hf) in enumerate(spans):
                t0 = s0 // 128
                nt = ns // 128
                pc_lo, pc_hi = span_pc[si]
                es_t = sb.tile([128, 2 * (SPAN_SLOTS // 16)], dt.int16, tag="es", bufs=2)
                nc.sync.dma_start(
                    es_t[:, 0 : 2 * (ns // 16)], i_eidx[:, 2 * (s0 // 16) : 2 * ((s0 + ns) // 16)]
                )
                xg = sb.tile([128, ST * 640], dt.bfloat16, tag="xg", bufs=2)
                xg3 = xg[:, 0 : nt * EW].rearrange("p (t w) -> p t w", w=EW)
                src_tbl = h_tbl if l == 0 else x_tbl
                nc.gpsimd.dma_gather(
                    xg3, src_tbl[:, 0:EW], es_t[:, 0 : ns // 16],
                    ns, ns, EW, elem_step=(128 if l == 0 else 640), single_packet=False,
                )
                aldg = sb.tile([128, ST * 128], dt.bfloat16, tag="aldg", bufs=2)
                aldg3 = aldg[:, 0 : nt * 128].rearrange("p (t w) -> p t w", w=128)
                nc.gpsimd.dma_gather(
                    aldg3, ald_tbl[hf * ALD_HALF : hf * ALD_HALF + ALD_HALF, :],
                    es_t[:, ns // 16 : 2 * (ns // 16)],
                    ns, ns, 128, single_packet=False,
                )
                # logits -> exp over the span
                zt = sb.tile([128, ST * 4], dt.float32, tag="zt", bufs=3)
                nc.vector.tensor_tensor(
                    out=zt[:, 0 : nt * H].rearrange("p (t h) -> p t h", h=H),
                    in0=xg3[:, :, ALOFF : ALOFF + H], in1=aldg3[:, :, 0:H], op=ALU.add,
                )
                z2 = sb.tile([128, ST * 4], dt.float32, tag="z2", bufs=3)
                nc.vector.tensor_scalar(out=z2[:, 0 : nt * H], in0=zt[:, 0 : nt * H],
                                        scalar1=NEG, scalar2=None, op0=ALU.mult)
                nc.vector.tensor_tensor(out=z2[:, 0 : nt * H], in0=zt[:, 0 : nt * H],
                                        in1=z2[:, 0 : nt * H], op=ALU.max)
                ex = sb.tile([128, ST * 4], dt.float32, tag="ex", bufs=3)
                nc.scalar.activation(ex[:, 0 : nt * H], z2[:, 0 : nt * H], ACTF.Exp)
                exb = sb.tile([128, ST * 4], dt.bfloat16, tag="exb", bufs=3)
                nc.vector.tensor_copy(exb[:, 0 : nt * H], ex[:, 0 : nt * H])
                for tl in range(nt):
                    t = t0 + tl
                    plist = pairs_of_tile[t]
                    if not plist:
                        continue
                    xgs = sb.tile([128, 512], dt.bfloat16, tag="xgs", bufs=4)
                    for h in range(H):
                        nc.vector.tensor_scalar(
                            out=xgs[:, h * VW : (h + 1) * VW],
                            in0=xg3[:, tl, h * VW : (h + 1) * VW] if PERHEAD else xg3[:, tl, 0:VW],
                            scalar1=ex[:, tl * H + h : tl * H + h + 1],
                            scalar2=None, op0=ALU.mult,
                        )
                    for pc in plist:
                        _, w, first, last = pairs[pc]
                        oh = sb.tile([128, 128], dt.bfloat16, tag="oh", bufs=6)
                        nc.vector.tensor_scalar(
                            out=oh[:], in0=iota_sb[:], scalar1=dstloc_sb[:, pc : pc + 1],
                            scalar2=None, op0=ALU.is_equal,
                        )
                        if first:
                            open_ps[w] = (
                                ps.tile([128, 512], dt.float32, tag="win", bufs=2, name="ps_win"),
                                ps.tile([128, 8], dt.float32, tag="small", name="ps_wd"),
                            )
                        ps_o, ps_d = open_ps[w]
                        nc.tensor.matmul(out=ps_o[:, 0:SCW], lhsT=oh[:], rhs=xgs[:, 0:SCW],
                                         start=first, stop=last)
                        nc.tensor.matmul(out=ps_d[:, 0:H], lhsT=oh[:],
                                         rhs=exb[:, tl * H : (tl + 1) * H],
                                         start=first, stop=last)
                        if last:
                            flush(w, open_ps.pop(w))

            assert not open_ps

        def make_pass1(l, ps_s, ps_q, exps):
            IN, OC, H, EW, AC, VW, PERHEAD = LCFG[l]
            acc_, rs_out = ACCS[l]
            SCW = H * VW
            nch = OC // 128
            XW = VW if not PERHEAD else OC   # own-value row width

            def pass1_chunk(kc):
                for g5 in range(kc * RB // GRP, (kc + 1) * RB // GRP):
                    # batched loads: RS rows + own value rows for 5 blocks
                    ld5 = sb.tile([128, GRP * AC], dt.bfloat16, tag="ld5", bufs=2, name="ld5_t")
                    nc.sync.dma_start(
                        ld5[:].rearrange("p (t w) -> p t w", w=AC),
                        rs_out[g5 * GRP * 128 : (g5 + 1) * GRP * 128, :].rearrange("(t p) w -> p t w", p=128),
                    )
                    xl5 = sb.tile([128, GRP * 512], dt.bfloat16, tag="xl5", bufs=2, name="xl5_t")
                    src_tbl = h_tbl if l == 0 else x_tbl
                    nc.sync.dma_start(
                        xl5[:, 0 : GRP * XW].rearrange("p (t w) -> p t w", w=XW),
                        src_tbl[g5 * GRP * 128 : (g5 + 1) * GRP * 128, 0:XW].rearrange("(t p) w -> p t w", p=128),
                    )
                    for j in range(GRP):
                        nt = g5 * GRP + j
                        blk = ld5[:, j * AC : (j + 1) * AC]
                        xblk = xl5[:, j * XW : (j + 1) * XW]
                        # denominator: den_rs + exp_self + eps -> masked reciprocal
                        d4 = sb.tile([128, 4], dt.float32, tag="d4", bufs=2)
                        nc.vector.tensor_scalar(out=d4[:, 0:H], in0=blk[:, SCW : SCW + H],
                                                scalar1=EPS_DEN, scalar2=None, op0=ALU.add)
                        nc.vector.tensor_tensor(out=d4[:, 0:H], in0=d4[:, 0:H],
                                                in1=exps[:, nt * 4 : nt * 4 + H], op=ALU.add)
                        r4 = sb.tile([128, 4], dt.float32, tag="r4", bufs=2)
                        nc.vector.reciprocal(r4[:, 0:H], d4[:, 0:H])
                        nc.vector.tensor_scalar(out=r4[:, 0:H], in0=r4[:, 0:H],
                                                scalar1=mask_sb[:, nt : nt + 1], scalar2=None, op0=ALU.mult)
                        # numerator: num_rs + exp_self * x_i, then divide
                        nsum = sb.tile([128, 512], dt.bfloat16, tag="nsum", bufs=2)
                        for h in range(H):
                            cs = slice(h * VW, (h + 1) * VW)
                            nc.vector.tensor_scalar(
                                out=nsum[:, cs],
                                in0=xblk[:, cs] if PERHEAD else xblk[:, 0:VW],
                                scalar1=exps[:, nt * 4 + h : nt * 4 + h + 1],
                                scalar2=None, op0=ALU.mult,
                            )
                        nc.vector.tensor_tensor(out=nsum[:, 0:SCW], in0=nsum[:, 0:SCW],
                                                in1=blk[:, 0:SCW], op=ALU.add)
                        if l == 0:
                            # divide in aggregated 256-space, then transform by W0
                            hga = sb.tile([128, 256], dt.bfloat16, tag="hga", bufs=2)
                            for h in range(H):
                                nc.vector.tensor_scalar(
                                    out=hga[:, h * VW : (h + 1) * VW], in0=nsum[:, h * VW : (h + 1) * VW],
                                    scalar1=r4[:, h : h + 1], scalar2=None, op0=ALU.mult,
                                )
                            haT = sb.tile([EMB, 4 * 128], dt.bfloat16, tag="haT", bufs=2)
                            for h in range(H):
                                ps_tr = ps.tile([EMB, 128], dt.bfloat16, tag="med", name="ps_tr")
                                nc.tensor.matmul(out=ps_tr[:], lhsT=hga[:, h * VW : (h + 1) * VW],
                                                 rhs=idenb_sb[:], is_transpose=True, start=True, stop=True)
                                nc.vector.tensor_copy(haT[:, h * 128 : (h + 1) * 128], ps_tr[:])
                            ps_x = ps.tile([128, 512], dt.float32, tag="win", bufs=2, name="ps_p1x")
                            for h in range(H):
                                nc.tensor.matmul(
                                    out=ps_x[:, h * 128 : (h + 1) * 128],
                                    lhsT=haT[:, h * 128 : (h + 1) * 128],
                                    rhs=wx_sb[0][0][:, h * 128 : (h + 1) * 128],
                                    start=True, stop=True,
                                )
                            hblk = sb.tile([128, 512], dt.bfloat16, tag="hblk", bufs=2)
                            nc.scalar.activation(hblk[:, 0:OC], ps_x[:, 0:OC], ACTF.Copy)
                        else:
                            hblk = sb.tile([128, 512], dt.bfloat16, tag="hblk", bufs=2)
                            for h in range(H):
                                cs = slice(h * VW, (h + 1) * VW)
                                nc.vector.tensor_scalar(
                                    out=hblk[:, cs], in0=nsum[:, cs],
                                    scalar1=r4[:, h : h + 1], scalar2=None, op0=ALU.mult,
                                )
                        sqb = sb.tile([128, 512], dt.bfloat16, tag="sqb", bufs=1)
                        nc.vector.tensor_tensor(out=sqb[:, 0:OC], in0=hblk[:, 0:OC], in1=hblk[:, 0:OC], op=ALU.mult)
                        nc.tensor.matmul(out=ps_s[:], lhsT=onesb_sb[:], rhs=hblk[:, 0:OC],
                                         start=(nt == 0), stop=(nt == NBLK - 1))
                        nc.tensor.matmul(out=ps_q[:], lhsT=onesb_sb[:], rhs=sqb[:, 0:OC],
                                         start=(nt == 0), stop=(nt == NBLK - 1))
                        for kk in range(nch):
                            ps_t = ps.tile([128, 128], dt.bfloat16, tag="med")
                            nc.tensor.matmul(out=ps_t[:], lhsT=hblk[:, kk * 128 : (kk + 1) * 128],
                                             rhs=idenb_sb[:], is_transpose=True, start=True, stop=True)
                            if (nt + kk) % 2 == 0:
                                nc.scalar.activation(hTe[:, kk * N2 + nt * 128 : kk * N2 + (nt + 1) * 128], ps_t[:], ACTF.Copy)
                            else:
                                nc.vector.tensor_copy(hTe[:, kk * N2 + nt * 128 : kk * N2 + (nt + 1) * 128], ps_t[:])

            return pass1_chunk

        def post_phase(l, ps_s, ps_q):
            IN, OC, H, EW, AC, VW, PERHEAD = LCFG[l]
            nch = OC // 128
            # BN stats AllReduce -> affine coeffs
            gam_t = sb.tile([1, OC], dt.float32, tag="gamt", bufs=1)
            nc.sync.dma_start(gam_t[:], P[f"gam{l}"][0:1, 0:OC])
            bet_t = sb.tile([1, OC], dt.float32, tag="bett", bufs=1)
            nc.sync.dma_start(bet_t[:], P[f"bet{l}"][0:1, 0:OC])
            stats = sb.tile([1, 2 * OC], dt.float32, tag="stats", bufs=1)
            nc.vector.tensor_copy(stats[0:1, 0:OC], ps_s)
            nc.vector.tensor_copy(stats[0:1, OC : 2 * OC], ps_q)
            st_in = dram.tile([1, 2 * OC], dt.float32, tag="st_in")
            st_out = dram.tile([1, 2 * OC], dt.float32, tag=f"st_out{l}", addr_space="Shared", name=f"st_out{l}")
            nc.sync.dma_start(st_in[:], stats[:])
            nc.gpsimd.collective_compute(
                "AllReduce", ALU.add, ins=[st_in[:]], outs=[st_out[:]], replica_groups=[list(range(NC))]
            )
            st2 = sb.tile([1, 2 * OC], dt.float32, tag="stats", bufs=1)
            nc.sync.dma_start(st2[:], st_out[:])
            m = sb.tile([1, OC], dt.float32, tag="bn_m", bufs=1)
            q = sb.tile([1, OC], dt.float32, tag="bn_q", bufs=1)
            nc.vector.tensor_scalar(out=m[:], in0=st2[0:1, 0:OC], scalar1=1.0 / N_NODES, scalar2=None, op0=ALU.mult)
            nc.vector.tensor_scalar(out=q[:], in0=st2[0:1, OC : 2 * OC], scalar1=1.0 / N_NODES, scalar2=None, op0=ALU.mult)
            var = sb.tile([1, OC], dt.float32, tag="bn_v", bufs=1)
            nc.vector.tensor_tensor(out=var[:], in0=m[:], in1=m[:], op=ALU.mult)
            nc.vector.tensor_tensor(out=var[:], in0=q[:], in1=var[:], op=ALU.subtract)
            epsc = sb.tile([1, 1], dt.float32, tag="epsc")
            nc.vector.memset(epsc[:], EPS)
            sd = sb.tile([1, OC], dt.float32, tag="bn_sd", bufs=1)
            nc.scalar.activation(sd[:], var[:], ACTF.Sqrt, bias=epsc[0:1, 0:1])
            rs = sb.tile([1, OC], dt.float32, tag="bn_rs", bufs=1)
            nc.vector.reciprocal(rs[:], sd[:])
            s_row = sb.tile([1, OC], dt.float32, tag="bn_s", bufs=1)
            nc.vector.tensor_tensor(out=s_row[:], in0=rs[:], in1=gam_t[:], op=ALU.mult)
            b_row = sb.tile([1, OC], dt.float32, tag="bn_b", bufs=1)
            nc.vector.tensor_tensor(out=b_row[:], in0=m[:], in1=s_row[:], op=ALU.mult)
            nc.vector.tensor_tensor(out=b_row[:], in0=bet_t[:], in1=b_row[:], op=ALU.subtract)
            sbc = sb.tile([128, 2 * nch], dt.float32, tag="sbc")
            for k in range(nch):
                ps_c = ps.tile([128, 1], dt.float32, tag="small")
                nc.tensor.matmul(out=ps_c[:], lhsT=s_row[0:1, k * 128 : (k + 1) * 128], rhs=onesr_sb[0:1, 0:1], start=True, stop=True)
                nc.vector.tensor_copy(sbc[:, k : k + 1], ps_c[:])
                ps_c2 = ps.tile([128, 1], dt.float32, tag="small")
                nc.tensor.matmul(out=ps_c2[:], lhsT=b_row[0:1, k * 128 : (k + 1) * 128], rhs=onesr_sb[0:1, 0:1], start=True, stop=True)
                nc.vector.tensor_copy(sbc[:, nch + k : nch + k + 1], ps_c2[:])
            # affine + ELU in place on hTe, stripes of 4 blocks
            SW = 4
            for s0 in range(0, NBLK, SW):
                sw = min(SW, NBLK - s0)
                W = sw * 128
                for k in range(nch):
                    ystr = sb.tile([128, SW * 128], dt.bfloat16, tag="ystr", bufs=1)
                    nc.vector.tensor_scalar(
                        out=ystr[:, 0:W], in0=hTe[:, k * N2 + s0 * 128 : k * N2 + s0 * 128 + W],
                        scalar1=sbc[:, k : k + 1], scalar2=sbc[:, nch + k : nch + k + 1],
                        op0=ALU.mult, op1=ALU.add,
                    )
                    t1 = sb.tile([128, SW * 128], dt.bfloat16, tag="elu1", bufs=1)
                    nc.vector.tensor_scalar(out=t1[:, 0:W], in0=ystr[:, 0:W], scalar1=0.0, scalar2=None, op0=ALU.min)
                    e1 = sb.tile([128, SW * 128], dt.bfloat16, tag="elu2", bufs=1)
                    nc.scalar.activation(e1[:, 0:W], t1[:, 0:W], ACTF.Exp)
                    r1 = sb.tile([128, SW * 128], dt.bfloat16, tag="elu3", bufs=1)
                    nc.vector.tensor_scalar(out=r1[:, 0:W], in0=ystr[:, 0:W], scalar1=0.0, scalar2=-1.0, op0=ALU.max, op1=ALU.add)
                    nc.vector.tensor_tensor(
                        out=hTe[:, k * N2 + s0 * 128 : k * N2 + s0 * 128 + W],
                        in0=e1[:, 0:W], in1=r1[:, 0:W], op=ALU.add,
                    )

        for l in range(3):
            exps = x_phase(l)
            ps_s = ps.tile([1, 512], dt.float32, tag="row", name="ps_s")
            ps_q = ps.tile([1, 512], dt.float32, tag="row", name="ps_q")
            ps_s = ps_s[:, 0 : LCFG[l][1]]
            ps_q = ps_q[:, 0 : LCFG[l][1]]
            p1 = make_pass1(l, ps_s, ps_q, exps)
            edge_phase(l)
            for kc in range(K_CH):
                p1(kc)
            post_phase(l, ps_s, ps_q)

        # =========================================================
        # pooling + classifier
        # =========================================================
        meanT = sb1.tile([128, N_GRAPHS], dt.float32, tag="meanT")
        maxT = sb1.tile([128, N_GRAPHS], dt.float32, tag="maxT")
        nc.vector.memset(meanT[:], 0)
        nc.vector.memset(maxT[:], -1e30)
        mask8_sb = load_sb(i_mask8, [128, NC], dt.float32, "mask8")
        mask8n_sb = load_sb(i_mask8n, [128, NC], dt.float32, "mask8n")
        maxseg = max(bnd - a for lst in segs for (a, bnd, g, inv) in lst)
        for cc in range(NC):
            scrm = sb.tile([128, N_GRAPHS], dt.float32, tag="scrm", bufs=1)
            scrx = sb.tile([128, N_GRAPHS], dt.float32, tag="scrx", bufs=1)
            nc.vector.memset(scrm[:], 0)
            nc.vector.memset(scrx[:], -1e30)
            for (a, bnd, g, inv) in segs[cc]:
                r1 = sb.tile([128, 1], dt.float32, tag="segr", bufs=4)
                pscr = sb.tile([128, maxseg], dt.bfloat16, tag="poolscr", bufs=2)
                nc.scalar.activation(pscr[:, 0 : bnd - a], hTe[:, a:bnd], ACTF.Copy, accum_out=r1[:])
                nc.vector.tensor_scalar(out=scrm[:, g : g + 1], in0=r1[:], scalar1=inv, scalar2=None, op0=ALU.mult)
                nc.vector.tensor_reduce(out=scrx[:, g : g + 1], in_=hTe[:, a:bnd], axis=AXX, op=ALU.max)
            nc.vector.tensor_scalar(out=scrm[:], in0=scrm[:], scalar1=mask8_sb[:, cc : cc + 1], scalar2=None, op0=ALU.mult)
            nc.vector.tensor_tensor(out=meanT[:], in0=meanT[:], in1=scrm[:], op=ALU.add)
            nc.vector.tensor_scalar(
                out=scrx[:], in0=scrx[:], scalar1=mask8_sb[:, cc : cc + 1],
                scalar2=mask8n_sb[:, cc : cc + 1], op0=ALU.mult, op1=ALU.add,
            )
            nc.vector.tensor_tensor(out=maxT[:], in0=maxT[:], in1=scrx[:], op=ALU.max)
        pm_in = dram.tile([128, N_GRAPHS], dt.float32, tag="pm_in")
        pm_out = dram.tile([128, N_GRAPHS], dt.float32, tag="pm_out", addr_space="Shared")
        px_in = dram.tile([128, N_GRAPHS], dt.float32, tag="px_in")
        px_out = dram.tile([128, N_GRAPHS], dt.float32, tag="px_out", addr_space="Shared")
        nc.sync.dma_start(pm_in[:], meanT[:])
        nc.sync.dma_start(px_in[:], maxT[:])
        nc.gpsimd.collective_compute("AllReduce", ALU.add, ins=[pm_in[:]], outs=[pm_out[:]], replica_groups=[list(range(NC))])
        nc.gpsimd.collective_compute("AllReduce", ALU.max, ins=[px_in[:]], outs=[px_out[:]], replica_groups=[list(range(NC))])
        meanF, maxF = meanT, maxT
        nc.sync.dma_start(meanF[:], pm_out[:])
        nc.sync.dma_start(maxF[:], px_out[:])

        cw1t_sb = [None, None]
        cw2t_sb = [None, None]
        for k in range(2):
            cw1t_sb[k] = sb1.tile([128, GDIM], dt.float32, tag=f"cw1t{k}", name=f"cw1t{k}")
            nc.sync.dma_start(cw1t_sb[k][:], P["cw1t"][k * 128 : (k + 1) * 128, :])
            cw2t_sb[k] = sb1.tile([128, NUM_CLASSES], dt.float32, tag=f"cw2t{k}", name=f"cw2t{k}")
            nc.sync.dma_start(cw2t_sb[k][:], P["cw2t"][k * 128 : (k + 1) * 128, :])
        cb1c_sb = load_sb(P["cb1c"], [128, 2], dt.float32, "cb1c")
        cb2c_sb = load_sb(P["cb2c"], [NUM_CLASSES, 1], dt.float32, "cb2c")
        hidT = sb1.tile([128, 2 * N_GRAPHS], dt.float32, tag="hidT")
        for hc in range(2):
            ps_h = ps.tile([128, N_GRAPHS], dt.float32, tag="med", name="ps_h")
            for dc, embT in enumerate([meanF, maxF]):
                nc.tensor.matmul(
                    out=ps_h[:], lhsT=cw1t_sb[dc][:, hc * 128 : (hc + 1) * 128],
                    rhs=embT[:], start=(dc == 0), stop=(dc == 1),
                )
            nc.vector.tensor_scalar(
                out=hidT[:, hc * N_GRAPHS : (hc + 1) * N_GRAPHS], in0=ps_h[:],
                scalar1=cb1c_sb[:, hc : hc + 1], scalar2=0.0, op0=ALU.add, op1=ALU.max,
            )
        ps_o = ps.tile([NUM_CLASSES, N_GRAPHS], dt.float32, tag="med", name="ps_out")
        for hc in range(2):
            nc.tensor.matmul(
                out=ps_o[:], lhsT=cw2t_sb[hc][:],
                rhs=hidT[:, hc * N_GRAPHS : (hc + 1) * N_GRAPHS], start=(hc == 0), stop=(hc == 1),
            )
        osb = sb1.tile([NUM_CLASSES, N_GRAPHS], dt.float32, tag="osb")
        nc.vector.tensor_scalar(out=osb[:], in0=ps_o[:], scalar1=cb2c_sb[:], scalar2=None, op0=ALU.add)
        for gc in range(2):
            ps_tt = ps.tile([128, NUM_CLASSES], dt.float32, tag="med", name="ps_tt")
            nc.tensor.matmul(
                out=ps_tt[:], lhsT=osb[:, gc * 128 : (gc + 1) * 128],
                rhs=idenf_sb[0:NUM_CLASSES, 0:NUM_CLASSES], start=True, stop=True,
            )
            ot = sb1.tile([128, NUM_CLASSES], dt.float32, tag="ot")
            nc.vector.tensor_copy(ot[:], ps_tt[:])
            nc.sync.dma_start(out_dram[gc * 128 : (gc + 1) * 128, :], ot[:])

    nc.compile()
    return nc


def kernel(**inputs):
    import concourse.bass_utils as bass_utils
    import hashlib

    pre = preprocess(inputs["x"], inputs["edge_index"], inputs["depth"], inputs["batch"])
    pb = build_param_blobs(inputs)

    sch = pre["sched"]
    sig = hashlib.sha1(
        sch["slots_w"].tobytes() + repr(sch["spans"]).encode() + repr(sch["segs"]).encode()
    ).hexdigest()
    if _CACHE.get("sig") != sig:
        _CACHE["built"] = build_nc(pre)
        _CACHE["sig"] = sig
    nc = _CACHE["built"]

    in_maps = []
    for c in range(NC):
        b = pre["blobs"][c]
        m = dict(
            eidx=b["eidx"], dstloc=b["dstloc"],
            emb_idx=b["emb_idx"], depthc=b["depthc"],
            mask8=b["mask8"], mask8n=b["mask8n"], maskcol=b["maskcol"],
        )
        m.update(pb)
        in_maps.append(m)

    import os, time

    trace = bool(int(os.environ.get("KERNEL_TRACE", "0")))
    t0 = time.time()
    res = bass_utils.run_bass_kernel_spmd(
        nc, in_maps, core_ids=list(range(NC)), trace=trace
    )
    _CACHE["run_s"] = time.time() - t0
    _CACHE["last_results"] = res
    return np.asarray(res.results[0]["out"], dtype=np.float32)


if __name__ == "__main__":
    sys.path.insert(0, "/root/problem")
    import reference

    inp = {k: np.asarray(v) for k, v in reference.setup_inputs().items()}
    got = kernel(**inp)
    exp = np.asarray(reference.reference(**inp))
    err = np.abs(got - exp).max() / (np.abs(exp).max() + 1e-30)
    print("Relative error:", err)
